# revision 1
# baseline (speedup 1.0000x reference)
"""DCGRU cell on 8 Trainium2 NeuronCores.

Sharding: data-parallel over batch (B=32 -> 4 per core), adjacency + MLP
weights replicated. No collectives; host gathers per-core outputs.

Per-core layouts (all f32):
  node-major (nm): [16 tiles][128 nodes, 768] cols = b*192+f   (diffusion lhsT)
  feat-major (fm): [6 tiles][128 bf-rows, 2048 nodes]          (hop outputs, MLP rhs)
Hop matmul: out_fm[bf, i] = sum_j x_nm[j, bf] * W[i, j]
  = matmul(lhsT=x_nm[jt][:, c*128:+128], rhs=WT[jt][:, i-block]) accumulated
  over jt in PSUM, so W is streamed host-pretransposed (WT[j, i] = W[i, j]).
MLP: gate logits acc[b][o, n] += WxI[k][bf, o].T @ fm[k][bf-slice, n] with
  batch-interleaved host-packed weights WxI (rows = b*192+f), accumulated
  across hops in DRAM via accum_op=add DMAs straight from PSUM.
Chain re-entry: fm -> nm via PE transposes (hops 1,2 of each direction only).
"""

import sys
import numpy as np
import ml_dtypes

for _p in ("/opt/trn_rl_repo",):
    if _p not in sys.path:
        sys.path.insert(0, _p)

from concourse import bacc, tile, mybir  # noqa: E402
from concourse.alu_op_type import AluOpType as ALU  # noqa: E402
from concourse.bass_utils import run_bass_kernel_spmd  # noqa: E402

F32 = mybir.dt.float32
F32R = mybir.dt.float32r
BF16 = mybir.dt.bfloat16
MM_BF16 = True          # matmul datapath dtype: True -> bf16, False -> f32r
MMDT = BF16 if MM_BF16 else F32R
AF = mybir.ActivationFunctionType

C = 4          # batches per core
FI = 192       # per-batch feature width (x 64 + h 128)
BF = C * FI    # 768
DH = 128
NCORES = 8
NHOPS = 3


def build_nc(nt=16):
    """Build + compile the per-core Bass kernel. nt = node tiles (N = nt*128)."""
    N = nt * 128
    nbk = N // 512

    nc = bacc.Bacc("TRN2", target_bir_lowering=False, debug=False,
                   num_devices=NCORES)

    def din(name, shape, dt=F32):
        return nc.dram_tensor(name, shape, dt, kind="ExternalInput").ap()

    XH = din("xh_nm", [nt, 128, BF], MMDT)
    XHFM = din("xh_fm", [6, 128, N], MMDT)
    WFT = din("wfT", [nt, 128, N], MMDT)
    WBT = din("wbT", [nt, 128, N], MMDT)
    WRI = din("wrI", [7, 3, 64, 128], MMDT)
    WZI = din("wzI", [7, 3, 64, 128], MMDT)
    WNI = din("wnI", [7, 3, 64, 128], MMDT)
    XFM = din("x_fm", [C, 64, N], MMDT)
    HFM = din("h_fm", [C, 128, N])
    BR = din("br_c", [128, 1])
    BZ = din("bz_c", [128, 1])
    BN = din("bn_c", [128, 1])
    IDT = din("ident", [128, 128], MMDT)
    OUT = nc.dram_tensor("out_fm", [C, 128, N], F32, kind="ExternalOutput").ap()

    ACCR = nc.dram_tensor("acc_r", [C, 128, N], F32).ap()
    ACCZ = nc.dram_tensor("acc_z", [C, 128, N], F32).ap()
    ACCN = nc.dram_tensor("acc_n", [C, 128, N], F32).ap()
    XRH = nc.dram_tensor("xrh_nm_d", [nt, 128, BF], MMDT).ap()

    with tile.TileContext(nc) as tc:
        with (
            tc.tile_pool(name="nm", bufs=32) as nm_pool,
            tc.tile_pool(name="fm", bufs=12) as fm_pool,
            tc.tile_pool(name="gate", bufs=4) as gate_pool,
            tc.tile_pool(name="wt", bufs=6) as wt_pool,
            tc.tile_pool(name="wxi", bufs=18) as wxi_pool,
            tc.tile_pool(name="aux", bufs=12) as aux_pool,
            tc.tile_pool(name="stg", bufs=4) as stg_pool,
            tc.tile_pool(name="const", bufs=1) as const_pool,
            tc.tile_pool(name="ps", bufs=6, space="PSUM") as ps_pool,
            tc.tile_pool(name="psx", bufs=2, space="PSUM") as psx_pool,
        ):
            ident = const_pool.tile([128, 128], MMDT, tag="ident")
            nc.sync.dma_start(ident[:], IDT[:])
            brt = const_pool.tile([128, 1], F32, tag="brt")
            nc.sync.dma_start(brt[:], BR[:])
            bzt = const_pool.tile([128, 1], F32, tag="bzt")
            nc.sync.dma_start(bzt[:], BZ[:])
            bnt = const_pool.tile([128, 1], F32, tag="bnt")
            nc.sync.dma_start(bnt[:], BN[:])

            def load_nm(SRC):
                ts = []
                for jt in range(nt):
                    t = nm_pool.tile([128, BF], MMDT, name="nmt", tag="nm")
                    nc.sync.dma_start(t[:], SRC[jt])
                    ts.append(t)
                return ts

            def hop(src, WT):
                """One diffusion hop; returns fm tiles (6 x [128, N])."""
                fms = [fm_pool.tile([128, N], MMDT, name="fmt", tag="fm") for _ in range(6)]
                for ibk in range(nbk):
                    pss = [ps_pool.tile([128, 512], F32, name="pst", tag="ps")
                           for _ in range(6)]
                    for jt in range(nt):
                        wt = wt_pool.tile([128, 512], MMDT, name="wtt", tag="wt")
                        nc.sync.dma_start(
                            wt[:], WT[jt][:, 512 * ibk:512 * (ibk + 1)])
                        for c in range(6):
                            nc.tensor.matmul(
                                pss[c][:],
                                src[jt][:, 128 * c:128 * (c + 1)],
                                wt[:],
                                start=(jt == 0), stop=(jt == nt - 1))
                    for c in range(6):
                        nc.vector.tensor_copy(
                            fms[c][:, 512 * ibk:512 * (ibk + 1)], pss[c][:])
                return fms

            def aux_of(fms):
                """Base-0 copies of rows [64:128) of each fm tile (so every
                MLP contraction segment sits at partition 0 -> one PSUM
                accumulation group, no mixed tile_position)."""
                auxs = []
                for t in range(6):
                    a = aux_pool.tile([64, N], MMDT, name="auxt", tag="aux")
                    nc.gpsimd.dma_start(a[:], fms[t][64:128, :])
                    auxs.append(a)
                return auxs

            def mlp_feed(fms, auxs, kidx, gates, first):
                """gates: list of (WXI dram, ACC dram). Accumulate logits."""
                for WXI, ACCD in gates:
                    wx = []
                    for s in range(3):
                        w = wxi_pool.tile([64, 128], MMDT, name="wxit", tag="wxi")
                        nc.gpsimd.dma_start(w[:], WXI[kidx][s])
                        wx.append(w)
                    for b in range(C):
                        for nb in range(nbk):
                            nbs = slice(512 * nb, 512 * (nb + 1))
                            ps = psx_pool.tile([128, 512], F32, name="psxt", tag="psx")
                            for s in range(3):
                                t, off = divmod(b * FI + 64 * s, 128)
                                rhs = (fms[t][0:64, nbs] if off == 0
                                       else auxs[t][0:64, nbs])
                                nc.tensor.matmul(ps[:], wx[s][:], rhs,
                                                 start=(s == 0), stop=(s == 2))
                            stg = stg_pool.tile([128, 512], F32, name="stgt", tag="stg")
                            nc.vector.tensor_copy(stg[:], ps[:])
                            nc.gpsimd.dma_start(
                                ACCD[b][:, nbs], stg[:],
                                accum_op=(ALU.bypass if first else ALU.add))

            def retranspose(fms):
                """fm tiles -> fresh nm tiles via PE transposes."""
                nms = [nm_pool.tile([128, BF], MMDT, name="nmt", tag="nm")
                       for _ in range(nt)]
                for it in range(nt):
                    ps = psx_pool.tile([128, BF], MMDT, name="psxt", tag="psx")
                    for c in range(6):
                        nc.tensor.transpose(
                            ps[:, 128 * c:128 * (c + 1)],
                            fms[c][:, 128 * it:128 * (it + 1)],
                            ident[:])
                    nc.vector.tensor_copy(nms[it][:], ps[:])
                return nms

            def diffusion(x_nm_loader, x_fm_tiles, gates, xnm_first=None):
                """Full 2-direction diffusion + MLP accumulation.
                MLP feeds are deferred one hop so they never gate the next
                hop's matmul stream (fm pool holds 2 chunks)."""
                mlp_feed(x_fm_tiles, aux_of(x_fm_tiles), 0, gates,
                         first=True)
                pending = None
                cur = xnm_first if xnm_first is not None else x_nm_loader()
                for wdir, WT in ((0, WFT), (1, WBT)):
                    if wdir == 1:
                        cur = x_nm_loader()
                    for k in range(1, NHOPS + 1):
                        fm = hop(cur, WT)
                        aux = aux_of(fm)
                        cur = retranspose(fm) if k < NHOPS else None
                        if pending is not None:
                            mlp_feed(*pending)
                        pending = (fm, aux, wdir * NHOPS + k, gates, False)
                mlp_feed(*pending)

            # ---------------- diffusion 1 (r, z gates) ----------------
            fm0 = []
            for t in range(6):
                f = fm_pool.tile([128, N], MMDT, name="fmt", tag="fm")
                nc.scalar.dma_start(f[:], XHFM[t])
                fm0.append(f)
            diffusion(lambda: load_nm(XH), fm0, [(WRI, ACCR), (WZI, ACCZ)])

            # ------------- gates r, z; assemble xrh (nm + fm) -------------
            xrh_nm = [nm_pool.tile([128, BF], MMDT, name="nmt", tag="nm")
                      for _ in range(nt)]
            xrh_fm = [fm_pool.tile([128, N], MMDT, name="fmt", tag="fm") for _ in range(6)]
            for b in range(C):
                accr = gate_pool.tile([128, N], F32, name="gatet", tag="gate")
                nc.scalar.dma_start(accr[:], ACCR[b])
                r = gate_pool.tile([128, N], F32, name="gatet", tag="gate")
                nc.scalar.activation(r[:], accr[:], AF.Sigmoid, bias=brt[:])
                h = gate_pool.tile([128, N], F32, name="gatet", tag="gate")
                nc.scalar.dma_start(h[:], HFM[b])
                rh = fm_pool.tile([128, N], MMDT, name="fmt", tag="fm")
                nc.vector.tensor_mul(rh[:], r[:], h[:])
                # rh columns of xrh_nm (PE transpose 128-blocks)
                for g in range(nt // 4):
                    ps = psx_pool.tile([128, 512], MMDT, name="psxt", tag="psx")
                    for q in range(4):
                        it = 4 * g + q
                        nc.tensor.transpose(
                            ps[:, 128 * q:128 * (q + 1)],
                            rh[:, 128 * it:128 * (it + 1)], ident[:])
                    for q in range(4):
                        nc.vector.tensor_copy(
                            xrh_nm[4 * g + q][:, b * FI + 64:(b + 1) * FI],
                            ps[:, 128 * q:128 * (q + 1)])
                # fm rows of xrh: x piece then two rh 64-row pieces
                t, off = divmod(b * FI, 128)
                nc.scalar.dma_start(xrh_fm[t][off:off + 64, :], XFM[b])
                for s2 in range(2):
                    t, off = divmod(b * FI + 64 + 64 * s2, 128)
                    nc.scalar.dma_start(xrh_fm[t][off:off + 64, :],
                                        rh[64 * s2:64 * (s2 + 1), :])
            # x columns of xrh_nm straight from the xh param
            for jt in range(nt):
                for b in range(C):
                    nc.scalar.dma_start(xrh_nm[jt][:, b * FI:b * FI + 64],
                                        XH[jt][:, b * FI:b * FI + 64])
            # spill xrh_nm for the backward-chain reload
            for jt in range(nt):
                nc.sync.dma_start(XRH[jt], xrh_nm[jt][:])

            # ---------------- diffusion 2 (n gate) ----------------
            diffusion(lambda: load_nm(XRH), xrh_fm, [(WNI, ACCN)],
                      xnm_first=xrh_nm)

            # ---------------- final gate ----------------
            for b in range(C):
                accn = gate_pool.tile([128, N], F32, name="gatet", tag="gate")
                nc.scalar.dma_start(accn[:], ACCN[b])
                n_t = gate_pool.tile([128, N], F32, name="gatet", tag="gate")
                nc.scalar.activation(n_t[:], accn[:], AF.Tanh, bias=bnt[:])
                h = gate_pool.tile([128, N], F32, name="gatet", tag="gate")
                nc.scalar.dma_start(h[:], HFM[b])
                accz = gate_pool.tile([128, N], F32, name="gatet", tag="gate")
                nc.scalar.dma_start(accz[:], ACCZ[b])
                z = gate_pool.tile([128, N], F32, name="gatet", tag="gate")
                nc.scalar.activation(z[:], accz[:], AF.Sigmoid, bias=bzt[:])
                d = gate_pool.tile([128, N], F32, name="gatet", tag="gate")
                nc.vector.tensor_sub(d[:], n_t[:], h[:])
                zd2 = gate_pool.tile([128, N], F32, name="gatet", tag="gate")
                nc.vector.tensor_mul(zd2[:], z[:], d[:])
                o = gate_pool.tile([128, N], F32, name="gatet", tag="gate")
                nc.vector.tensor_add(o[:], zd2[:], h[:])
                nc.scalar.dma_start(OUT[b], o[:])

    nc.compile()
    return nc


def _pack_interleaved(W):
    """[128, 7*192] torch-Linear weight -> [7, 3, 64, 128] transposed 64-row
    contraction segments: out[k, s, f, o] = W[o, k*192 + 64*s + f]."""
    out = np.zeros((7, 3, 64, 128), np.float32)
    for k in range(7):
        for s in range(3):
            out[k, s] = W[:, k * FI + 64 * s:k * FI + 64 * (s + 1)].T
    return np.ascontiguousarray(out)


_NC_CACHE = {}


def _get_nc(nt):
    if nt not in _NC_CACHE:
        _NC_CACHE[nt] = build_nc(nt)
    return _NC_CACHE[nt]


def make_in_maps(x, h_prev, W_fwd, W_bwd, Wr, br, Wz, bz, Wn, bn):
    mdt = np.dtype(ml_dtypes.bfloat16) if MM_BF16 else np.float32
    x = np.asarray(x, np.float32)
    h_prev = np.asarray(h_prev, np.float32)
    B, N, Din = x.shape
    nt = N // 128
    WfT = np.ascontiguousarray(np.asarray(W_fwd, np.float32).T).reshape(nt, 128, N)
    WbT = np.ascontiguousarray(np.asarray(W_bwd, np.float32).T).reshape(nt, 128, N)
    wrI = _pack_interleaved(np.asarray(Wr, np.float32))
    wzI = _pack_interleaved(np.asarray(Wz, np.float32))
    wnI = _pack_interleaved(np.asarray(Wn, np.float32))
    ident = np.ascontiguousarray(np.eye(128, dtype=np.float32))
    WfT_d = WfT.astype(mdt)
    WbT_d = WbT.astype(mdt)
    wrI_d = wrI.astype(mdt)
    wzI_d = wzI.astype(mdt)
    wnI_d = wnI.astype(mdt)
    ident_d = ident.astype(mdt)
    brc = np.ascontiguousarray(np.asarray(br, np.float32).reshape(128, 1))
    bzc = np.ascontiguousarray(np.asarray(bz, np.float32).reshape(128, 1))
    bnc = np.ascontiguousarray(np.asarray(bn, np.float32).reshape(128, 1))
    ncores = B // C
    in_maps = []
    for cix in range(ncores):
        xs = x[C * cix:C * (cix + 1)]
        hs = h_prev[C * cix:C * (cix + 1)]
        xh = np.concatenate([xs, hs], axis=-1)            # [C, N, 192]
        flat = np.ascontiguousarray(xh.transpose(1, 0, 2).reshape(N, BF))
        xh_nm = np.ascontiguousarray(flat).reshape(nt, 128, BF)
        xh_fm = np.ascontiguousarray(flat.T).reshape(6, 128, N)
        x_fm = np.ascontiguousarray(xs.transpose(0, 2, 1))
        h_fm = np.ascontiguousarray(hs.transpose(0, 2, 1))
        in_maps.append(dict(
            xh_nm=xh_nm.astype(mdt), xh_fm=xh_fm.astype(mdt),
            wfT=WfT_d, wbT=WbT_d, wrI=wrI_d, wzI=wzI_d, wnI=wnI_d,
            x_fm=x_fm.astype(mdt), h_fm=h_fm,
            br_c=brc, bz_c=bzc, bn_c=bnc, ident=ident_d))
    return in_maps, nt, ncores


def kernel(x, h_prev, W_fwd, W_bwd, Wr, br, Wz, bz, Wn, bn, _trace=False):
    in_maps, nt, ncores = make_in_maps(
        x, h_prev, W_fwd, W_bwd, Wr, br, Wz, bz, Wn, bn)
    nc = _get_nc(nt)
    res = run_bass_kernel_spmd(nc, in_maps, list(range(ncores)), trace=_trace)
    outs = [np.ascontiguousarray(res.results[c]["out_fm"].transpose(0, 2, 1))
            for c in range(ncores)]
    full = np.concatenate(outs, axis=0).astype(np.float32)
    if _trace:
        return full, res
    return full



# revision 35
# speedup vs baseline: 4.6924x; 4.6924x over previous
"""DCGRU cell on 8 Trainium2 NeuronCores.

Sharding: data-parallel over batch (B=32 -> 4 per core), adjacency + MLP
weights replicated. No collectives; host gathers per-core outputs.

Key structure (all matmuls fp8 e4m3, DoubleRow perf mode = 2 contraction
subtiles per instruction):
  - W_fwd/W_bwd cached in SBUF as WT fp8 tiles [128, 16, 2048] scaled
    x1024 (row-stochastic entries ~1e-3 are subnormal in e4m3 otherwise).
  - Diffusion 1 (r,z gates): feature-major hop outputs fm2 [128, 6, N]
    with planes [h_b0 | x_b0;x_b1 | h_b1 | h_b2 | x_b2;x_b3 | h_b3]; batch
    b's 192 features = plane pair (pb, pb+1), pb = [0,1,3,4][b]. MLP feed
    = ONE DoubleRow matmul per (gate, chunk, b, nblk) with host-packed
    weight variants (x-rows zero-padded to match the shared x-pair
    plane). Logits accumulate in SBUF bf16 at 32x scale; sigmoid applies
    scale=2^-5. Chain re-entry fm->nm via PE transposes (hops 1,2).
  - Diffusion 2 (n gate): Horner chain T = W(y1 + W(y2 + W y3)) over
    node-major state t [128, 16, 512] (cols = b*128+o). Projections
    y_k = Wn_k^T x_rh are extra matmuls accumulated into the same PSUM
    group (x_rh feature-major as stationary operand). No transposes, no
    DRAM spill. n-logits land node-major, transposed once at the end.
  - Scale chain: hop-1 stores x2^-7 (=8x true), hops 2-3 x2^-10 (=8x),
    d1 MLP weights x32 (chunk 0) / x4 (chunks 1-6); d2 weights x1024,
    chain copies 2^-10, final tanh scale 2^-10.
  - ~9 DMAs total per core (DMA dispatch, not bandwidth, dominated the
    old design).
"""

import sys
import numpy as np
import ml_dtypes

for _p in ("/opt/trn_rl_repo",):
    if _p not in sys.path:
        sys.path.insert(0, _p)

from concourse import bacc, tile, mybir  # noqa: E402
from concourse.bass_utils import run_bass_kernel_spmd  # noqa: E402

F32 = mybir.dt.float32
BF16 = mybir.dt.bfloat16
FP8 = mybir.dt.float8e4
AF = mybir.ActivationFunctionType
DR = mybir.MatmulPerfMode.DoubleRow
NP_FP8 = ml_dtypes.float8_e4m3

C = 4          # batches per core
FI = 192       # per-batch feature width (x 64 + h 128)
BF = C * FI    # 768
DH = 128
NCORES = 8
NHOPS = 3

W2P = [0, 2, 3, 5, 1, 4]   # hop psum window -> fm2 plane
PB = [0, 1, 3, 4]          # batch -> first fm2 plane of its (lo, hi) pair

WSCALE = 1024.0            # W_fwd/W_bwd host prescale
HOP1_SCALE = 2.0 ** -7     # psum -> fm2 store, hop 1 (keeps feats at 8x)
HOPK_SCALE = 2.0 ** -10    # psum -> fm2 store, hops 2+
GATE_SCALE = 2.0 ** -5     # r/z logits accumulate at 32x
CHAIN_SCALE = 2.0 ** -10   # d2 chain psum -> t store / final tanh


def build_nc(nt=16):
    """Build + compile the per-core Bass kernel. nt = node tiles (N = nt*128)."""
    N = nt * 128
    npair = nt // 2

    nc = bacc.Bacc("TRN2", target_bir_lowering=False, debug=False,
                   num_devices=NCORES)

    def din(name, shape, dt=FP8):
        return nc.dram_tensor(name, shape, dt, kind="ExternalInput").ap()

    WF = din("wfT", [128, nt, N])
    WB = din("wbT", [128, nt, N])
    XNM = din("x_nm", [128, nt, BF])
    FM0 = din("fm0", [128, 6, N])
    HB = din("h_fm", [128, C, N], BF16)
    WX1 = din("wx1", [128, 56, 128])       # ((g*7+k)*2+v)*2 -> 2 planes
    WX2 = din("wx2", [128, 32, 128])       # (k*2+v)*2 -> 2 planes; 28-31 dW0 res
    WXB = din("wxb", [128, 2, 128], BF16)  # 32*Wr0_h.T | 32*Wz0_h.T
    IDT = din("ident", [128, 128])
    IDTB = din("identb", [128, 128], BF16)
    BIAS = din("bias", [128, 4], F32)      # br | bz | bn | -bz
    OUT = nc.dram_tensor("out_fm", [C, 128, N], F32, kind="ExternalOutput").ap()

    with tile.TileContext(nc) as tc:
        with (
            tc.tile_pool(name="w", bufs=4) as w_pool,
            tc.tile_pool(name="xnm", bufs=2) as xnm_pool,
            tc.tile_pool(name="fm", bufs=3) as fm_pool,
            tc.tile_pool(name="fm0", bufs=1) as fm0_pool,
            tc.tile_pool(name="acc", bufs=2) as acc_pool,
            tc.tile_pool(name="h", bufs=1) as h_pool,
            tc.tile_pool(name="wx", bufs=1) as wx_pool,
            tc.tile_pool(name="gate", bufs=1) as gate_pool,
            tc.tile_pool(name="sg", bufs=3) as sg_pool,
            tc.tile_pool(name="const", bufs=1) as const_pool,
            tc.tile_pool(name="psh", bufs=4, space="PSUM") as psh_pool,
            tc.tile_pool(name="psm", bufs=2, space="PSUM") as psm_pool,
            tc.tile_pool(name="pst", bufs=2, space="PSUM") as pst_pool,
        ):
            # ---- one-time loads (ordered so PE can start ASAP) ----
            fm0 = fm0_pool.tile([128, 6, N], FP8, name="fm0_t", tag="fm0")
            nc.sync.dma_start(fm0[:], FM0[:])
            wx1 = wx_pool.tile([128, 56 + 32, 128], FP8, name="wx_t", tag="wx")
            nc.sync.dma_start(wx1[:, 0:56, :], WX1[:])
            nc.sync.dma_start(wx1[:, 56:88, :], WX2[:])
            wxb = const_pool.tile([128, 2, 128], BF16, name="wxb_t", tag="wxb")
            nc.sync.dma_start(wxb[:], WXB[:])
            hb = h_pool.tile([128, C, N], BF16, name="h_t", tag="h")
            nc.sync.dma_start(hb[:], HB[:])
            cur = xnm_pool.tile([128, nt, BF], FP8, name="xnm_t", tag="xnm")
            nc.sync.dma_start(cur[:], XNM[:])
            hnt = nt // 2
            wf = [w_pool.tile([128, hnt, N], FP8, name="wt_t", tag="w")
                  for _ in range(2)]
            wb = [w_pool.tile([128, hnt, N], FP8, name="wt_t", tag="w")
                  for _ in range(2)]
            for hx in range(2):
                nc.sync.dma_start(wf[hx][:], WF[:, hnt * hx:hnt * (hx + 1), :])
            for hx in range(2):
                nc.sync.dma_start(wb[hx][:], WB[:, hnt * hx:hnt * (hx + 1), :])
            ident = const_pool.tile([128, 128], FP8, name="ident_t", tag="ident")
            nc.sync.dma_start(ident[:], IDT[:])
            identb = const_pool.tile([128, 128], BF16, name="identb_t", tag="identb")
            nc.sync.dma_start(identb[:], IDTB[:])
            bias = const_pool.tile([128, 4], F32, name="bias_t", tag="bias")
            nc.sync.dma_start(bias[:], BIAS[:])

            accr = acc_pool.tile([128, C, N], BF16, name="acc_t", tag="acc")
            accz = acc_pool.tile([128, C, N], BF16, name="acc_t", tag="acc")

            def hop_mm(ps, wt2, xnm2, ccols, ibs):
                """Full-contraction DR group: out[ccols-window, ibs]."""
                for jp in range(npair):
                    h2, j2 = divmod(2 * jp, hnt)
                    nc.tensor.matmul(
                        ps[:], xnm2[:, 2 * jp:2 * jp + 2, ccols],
                        wt2[h2][:, j2:j2 + 2, ibs],
                        start=(jp == 0), stop=(jp == npair - 1),
                        perf_mode=DR)

            def mlp_feed_b(srcs, b, g, acc):
                for ib in range(4):
                    nbs = slice(512 * ib, 512 * (ib + 1))
                    ps = psm_pool.tile([128, 512], F32, name="psm_t", tag="psm")
                    for s, (src, kidx) in enumerate(srcs):
                        widx = ((g * 7 + kidx) * 2 + (b & 1)) * 2
                        nc.tensor.matmul(
                            ps[:], wx1[:, widx:widx + 2, :],
                            src[:, PB[b]:PB[b] + 2, nbs],
                            start=(s == 0), stop=(s == len(srcs) - 1),
                            perf_mode=DR)
                    nc.vector.tensor_add(acc[:, b, nbs], ps[:],
                                         acc[:, b, nbs])

            def mlp_feed(srcs, init, gates, addeng=None):
                """srcs: list of (fm2, kidx) chunk pairs in one psum group.
                Accumulate bf16 logits; adds alternate DVE / Pool engines."""
                for g, acc in gates:
                    for b in range(C):
                        for ib in range(4):
                            nbs = slice(512 * ib, 512 * (ib + 1))
                            ps = psm_pool.tile([128, 512], F32, name="psm_t", tag="psm")
                            for s, (src, kidx) in enumerate(srcs):
                                widx = ((g * 7 + kidx) * 2 + (b & 1)) * 2
                                nc.tensor.matmul(
                                    ps[:], wx1[:, widx:widx + 2, :],
                                    src[:, PB[b]:PB[b] + 2, nbs],
                                    start=(s == 0), stop=(s == len(srcs) - 1),
                                    perf_mode=DR)
                            if init:
                                nc.vector.tensor_copy(acc[:, b, nbs], ps[:])
                            else:
                                nc.vector.tensor_add(acc[:, b, nbs], ps[:],
                                                     acc[:, b, nbs])

            # ---------------- diffusion 1 (r, z gates) ----------------
            # chunk-0 feed: h-part in bf16 (hb, wxb), x-part fp8 single plane
            for g, acc in ((0, accr), (1, accz)):
                for b in range(C):
                    widx = (g * 7 * 2 + (b & 1)) * 2
                    xw = widx + (0 if b & 1 else 1)
                    xpl = PB[b] if b & 1 else PB[b] + 1
                    for ib in range(4):
                        nbs = slice(512 * ib, 512 * (ib + 1))
                        ps = psm_pool.tile([128, 512], F32, name="psm_t", tag="psm")
                        nc.tensor.matmul(
                            ps[:], wxb[:, g, :], hb[:, b, nbs],
                            start=True, stop=False, skip_group_check=True)
                        nc.tensor.matmul(
                            ps[:], wx1[:, xw, :], fm0[:, xpl, nbs],
                            start=False, stop=True, skip_group_check=True)
                        nc.vector.tensor_copy(acc[:, b, nbs], ps[:])

            fm_hist = {}
            for wdir, wt2 in ((0, wf), (1, wb)):
                if wdir == 1:
                    cur = xnm_pool.tile([128, nt, BF], FP8, name="xnm_t", tag="xnm")
                    nc.sync.dma_start(cur[:], XNM[:])
                for k in range(1, NHOPS + 1):
                    cps = HOP1_SCALE if k == 1 else HOPK_SCALE
                    kidx = wdir * NHOPS + k
                    fm2 = fm_pool.tile([128, 6, N], FP8, name="fm_t", tag="fm")
                    fm_hist[kidx] = fm2
                    nxt = (xnm_pool.tile([128, nt, BF], FP8, name="xnm_t", tag="xnm")
                           if k < NHOPS else None)
                    for c in range(6):
                        ccols = slice(128 * c, 128 * (c + 1))
                        for ib in range(4):
                            ibs = slice(512 * ib, 512 * (ib + 1))
                            ps = psh_pool.tile([128, 512], F32, name="psh_t", tag="psh")
                            hop_mm(ps, wt2, cur, ccols, ibs)
                            last = (wdir, k) == (1, NHOPS)
                            eng = (nc.vector if last or (c * 4 + ib) & 1
                                   else nc.scalar)
                            if eng is nc.vector:
                                nc.vector.tensor_scalar_mul(
                                    fm2[:, W2P[c], ibs], ps[:], cps)
                            else:
                                nc.scalar.activation(
                                    fm2[:, W2P[c], ibs], ps[:], AF.Copy,
                                    scale=cps)
                            if nxt is not None:
                                # fm -> nm re-entry (it-blocks 4*ib..4*ib+3):
                                # transpose via plain fp8 matmul against the
                                # identity (fp8 transpose mode is rejected by
                                # the compiler)
                                pt = pst_pool.tile([128, 4, 128], F32,
                                                   name="pst_t", tag="pst")
                                for i in range(4):
                                    it = 4 * ib + i
                                    nc.tensor.matmul(
                                        pt[:, i, :],
                                        fm2[:, W2P[c], 128 * it:128 * (it + 1)],
                                        ident[:], start=True, stop=True)
                                nc.scalar.activation(
                                    nxt[:, 4 * ib:4 * ib + 4, ccols], pt[:],
                                    AF.Copy)
                    # feed chunk triples {1,2,3} / {4,5,6} once all exist
                    fk = {(0, 3): (1, 2, 3), (1, 3): (4, 5, 6)}.get((wdir, k))
                    if fk == (1, 2, 3):
                        mlp_feed([(fm_hist[j], j) for j in fk], False,
                                 [(0, accr), (1, accz)])
                    elif fk is not None:
                        # last round: per-batch r adds -> sigmoid -> r*h so
                        # the d2 chains can start before the z adds finish
                        srcs = [(fm_hist[j], j) for j in fk]
                        for b in range(C):
                            mlp_feed_b(srcs, b, 0, accr)
                            r = gate_pool.tile([128, N], BF16, name="gate_t",
                                               tag="gate")
                            nc.scalar.activation(r[:], accr[:, b, :],
                                                 AF.Sigmoid,
                                                 bias=bias[:, 0:1],
                                                 scale=GATE_SCALE)
                            nc.vector.tensor_mul(fm0[:, W2P[b], :], r[:],
                                                 hb[:, b, :])
                        for b in range(C):
                            mlp_feed_b(srcs, b, 1, accz)
                    if nxt is not None:
                        cur = nxt

            xrh2 = fm0

            # ---- z and u = (1-z)*h, computed while the chains run ----
            uh = [fm_pool.tile([128, 2, N], BF16, name="fm_t", tag="fm")
                  for _ in range(2)]
            for b in range(C):
                zc = gate_pool.tile([128, N], BF16, name="gate_t", tag="gate")
                nc.scalar.activation(zc[:], accz[:, b, :], AF.Sigmoid,
                                     bias=bias[:, 3:4], scale=-GATE_SCALE)
                nc.gpsimd.tensor_mul(uh[b // 2][:, b & 1, :], zc[:],
                                     hb[:, b, :])
            for b in range(C):
                nc.scalar.activation(accz[:, b, :], accz[:, b, :], AF.Sigmoid,
                                     bias=bias[:, 1:2], scale=GATE_SCALE)

            # ---------------- diffusion 2 (n gate): Horner chains ----------------
            accn = acc_pool.tile([128, nt, 512], BF16, name="acc_t", tag="acc")

            def proj_it(ps, it, kidx, start, stop):
                """y_k projection matmuls for one node block into psum ps."""
                for b in range(C):
                    widx = 56 + (kidx * 2 + (b & 1)) * 2
                    nc.tensor.matmul(
                        ps[:, 128 * b:128 * (b + 1)],
                        xrh2[:, PB[b]:PB[b] + 2, 128 * it:128 * (it + 1)],
                        wx1[:, widx:widx + 2, :],
                        start=start, stop=stop, perf_mode=DR,
                        skip_group_check=True)

            # y0 -> accn; extra DR matmul adds the fp8 weight residual
            for it in range(nt):
                ps = psh_pool.tile([128, 512], F32, name="psh_t", tag="psh")
                for b in range(C):
                    widx = 56 + (b & 1) * 2
                    rwidx = 84 + (b & 1) * 2
                    xs = xrh2[:, PB[b]:PB[b] + 2, 128 * it:128 * (it + 1)]
                    nc.tensor.matmul(
                        ps[:, 128 * b:128 * (b + 1)], xs, wx1[:, widx:widx + 2, :],
                        start=True, stop=False, perf_mode=DR,
                        skip_group_check=True)
                    nc.tensor.matmul(
                        ps[:, 128 * b:128 * (b + 1)], xs, wx1[:, rwidx:rwidx + 2, :],
                        start=False, stop=True, perf_mode=DR,
                        skip_group_check=True)
                nc.vector.tensor_copy(accn[:, it, :], ps[:])

            def final_gate_q(q):
                """out = u + z*tanh(n) for node block q (all batches)."""
                qs = slice(512 * q, 512 * (q + 1))
                for b in range(C):
                    pt = pst_pool.tile([128, 512], BF16, name="pst_t", tag="pst")
                    for i in range(4):
                        it = 4 * q + i
                        nc.tensor.transpose(
                            pt[:, 128 * i:128 * (i + 1)],
                            accn[:, it, 128 * b:128 * (b + 1)], identb[:])
                    n_t = sg_pool.tile([128, 512], F32, name="sg_t", tag="sg")
                    nc.scalar.activation(n_t[:], pt[:], AF.Tanh,
                                         bias=bias[:, 2:3], scale=CHAIN_SCALE)
                    zn = sg_pool.tile([128, 512], F32, name="sg_t", tag="sg")
                    nc.gpsimd.tensor_mul(zn[:], n_t[:], accz[:, b, qs])
                    og = sg_pool.tile([128, 512], F32, name="og_t", tag="sg")
                    nc.gpsimd.tensor_add(og[:], zn[:], uh[b // 2][:, b & 1, qs])
                    nc.sync.dma_start(OUT[b][:, qs], og[:])

            for wdir, wt2 in ((0, wf), (1, wb)):
                # t = y_3 (projection only)
                t = xnm_pool.tile([128, nt, 512], FP8, name="tch_t", tag="xnm")
                for it in range(nt):
                    ps = psh_pool.tile([128, 512], F32, name="psh_t", tag="psh")
                    proj_it(ps, it, wdir * NHOPS + NHOPS, True, True)
                    nc.vector.tensor_scalar_mul(t[:, it, :], ps[:], CHAIN_SCALE)
                for k in range(NHOPS, 0, -1):
                    # next state = W*t (+ y_{k-1} while k>1)
                    tn = (xnm_pool.tile([128, nt, 512], FP8, name="tch_t", tag="xnm")
                          if k > 1 else None)
                    for it in range(nt):
                        ccols = slice(128 * it, 128 * (it + 1))
                        ps = psh_pool.tile([128, 512], F32, name="psh_t", tag="psh")
                        for jp in range(npair):
                            h2, j2 = divmod(2 * jp, hnt)
                            nc.tensor.matmul(
                                ps[:], wt2[h2][:, j2:j2 + 2, ccols],
                                t[:, 2 * jp:2 * jp + 2, :],
                                start=(jp == 0), stop=(jp == npair - 1 and k == 1),
                                perf_mode=DR, skip_group_check=True)
                        if k > 1:
                            proj_it(ps, it, wdir * NHOPS + k - 1, False, True)
                            if it & 1:
                                nc.scalar.activation(tn[:, it, :], ps[:],
                                                     AF.Copy, scale=CHAIN_SCALE)
                            else:
                                nc.vector.tensor_scalar_mul(
                                    tn[:, it, :], ps[:], CHAIN_SCALE)
                        else:
                            nc.vector.tensor_add(accn[:, it, :], ps[:],
                                                 accn[:, it, :])
                            if wdir == 1 and it % 4 == 3:
                                final_gate_q(it // 4)
                    if k > 1:
                        t = tn

    nc.compile()
    return nc


def _pack_gate_variants(W, scales):
    """Torch-Linear weight [128, 7*192] -> [7*2, 2, 64... ] DR variants.

    For chunk k (feature slice [k*192:(k+1)*192] = [x(64) | h(128)]) emit
    variant A (even b: planes (lo=h, hi=x-upper)) and B (odd b: planes
    (x-lower, h)), each [2, 128, 128] with rows = contraction features and
    cols = output unit. Returns [7, 2, 2, 128, 128] float32.
    """
    out = np.zeros((7, 2, 2, 128, 128), np.float32)
    for k in range(7):
        s = scales[k]
        Wx = s * W[:, k * FI:k * FI + 64].T          # [64, 128]
        Wh = s * W[:, k * FI + 64:(k + 1) * FI].T    # [128, 128]
        out[k, 0, 0] = Wh
        out[k, 0, 1, 0:64] = Wx
        out[k, 1, 0, 64:128] = Wx
        out[k, 1, 1] = Wh
    return out


_NC_CACHE = {}


def _get_nc(nt):
    if nt not in _NC_CACHE:
        _NC_CACHE[nt] = build_nc(nt)
    return _NC_CACHE[nt]


def make_in_maps(x, h_prev, W_fwd, W_bwd, Wr, br, Wz, bz, Wn, bn):
    x = np.asarray(x, np.float32)
    h_prev = np.asarray(h_prev, np.float32)
    B, N, Din = x.shape
    nt = N // 128

    def to_pmajor(a):
        # [N(j), cols] -> [128(p), nt(jt), cols] with j = jt*128 + p
        cols = a.shape[1]
        return np.ascontiguousarray(
            a.reshape(nt, 128, cols).transpose(1, 0, 2))

    WfT = to_pmajor(np.asarray(W_fwd, np.float32).T * WSCALE).astype(NP_FP8)
    WbT = to_pmajor(np.asarray(W_bwd, np.float32).T * WSCALE).astype(NP_FP8)

    d1scales = [32.0] + [4.0] * 6
    wr_v = _pack_gate_variants(np.asarray(Wr, np.float32), d1scales)
    wz_v = _pack_gate_variants(np.asarray(Wz, np.float32), d1scales)
    wn_v = _pack_gate_variants(np.asarray(Wn, np.float32), [WSCALE] * 7)
    wn8 = wn_v.astype(NP_FP8).astype(np.float32)
    dwn0 = wn_v[0] - wn8[0]                      # chunk-0 weight fp8 residual
    # wx1 [128, 56, 128]: plane ((g*7+k)*2+v)*2 + pl
    wx1 = np.concatenate([wr_v, wz_v]).reshape(14, 2, 2, 128, 128)
    wx1 = np.ascontiguousarray(
        wx1.reshape(28, 2, 128, 128).reshape(56, 128, 128)
        .transpose(1, 0, 2)).astype(NP_FP8)
    wx2 = np.concatenate(
        [wn_v.reshape(28, 128, 128), dwn0.reshape(4, 128, 128)])
    wx2 = np.ascontiguousarray(wx2.transpose(1, 0, 2)).astype(NP_FP8)
    wxb = np.stack([32.0 * np.asarray(Wr, np.float32)[:, 64:192].T,
                    32.0 * np.asarray(Wz, np.float32)[:, 64:192].T])
    wxb = np.ascontiguousarray(
        wxb.transpose(1, 0, 2)).astype(ml_dtypes.bfloat16)

    ident8 = np.eye(128, dtype=np.float32).astype(NP_FP8)
    identb = np.eye(128, dtype=np.float32).astype(ml_dtypes.bfloat16)
    biases = np.stack([np.asarray(br, np.float32),
                       np.asarray(bz, np.float32),
                       np.asarray(bn, np.float32),
                       -np.asarray(bz, np.float32)], axis=1)  # [128, 4]
    biases = np.ascontiguousarray(biases)

    ncores = B // C
    in_maps = []
    for cix in range(ncores):
        xs = x[C * cix:C * (cix + 1)]            # [C, N, 64]
        hs = h_prev[C * cix:C * (cix + 1)]       # [C, N, 128]
        # x_nm cols: [h_b0|h_b1|h_b2|h_b3|x_b0|x_b1|x_b2|x_b3]
        xnm_cols = np.concatenate(
            [hs[b] for b in range(C)] + [xs[b] for b in range(C)], axis=1)
        x_nm = to_pmajor(xnm_cols).astype(NP_FP8)
        # fm0 planes: [h0.T | x0.T;x1.T | h1.T | h2.T | x2.T;x3.T | h3.T]
        hT = [np.ascontiguousarray(hs[b].T) for b in range(C)]
        xT = [np.ascontiguousarray(xs[b].T) for b in range(C)]
        fm0 = np.stack([
            hT[0], np.concatenate([xT[0], xT[1]], axis=0), hT[1],
            hT[2], np.concatenate([xT[2], xT[3]], axis=0), hT[3],
        ], axis=1)                               # [128, 6, N]
        h_fm = np.ascontiguousarray(
            np.stack(hT, axis=1)).astype(ml_dtypes.bfloat16)  # [128, C, N]
        in_maps.append(dict(
            wfT=WfT, wbT=WbT, x_nm=x_nm,
            fm0=np.ascontiguousarray(fm0).astype(NP_FP8),
            h_fm=h_fm, wx1=wx1, wx2=wx2, wxb=wxb,
            ident=ident8, identb=identb, bias=biases))
    return in_maps, nt, ncores


def kernel(x, h_prev, W_fwd, W_bwd, Wr, br, Wz, bz, Wn, bn, _trace=False):
    in_maps, nt, ncores = make_in_maps(
        x, h_prev, W_fwd, W_bwd, Wr, br, Wz, bz, Wn, bn)
    nc = _get_nc(nt)
    res = run_bass_kernel_spmd(nc, in_maps, list(range(ncores)), trace=_trace)
    outs = [np.ascontiguousarray(res.results[c]["out_fm"].transpose(0, 2, 1))
            for c in range(ncores)]
    full = np.concatenate(outs, axis=0).astype(np.float32)
    if _trace:
        return full, res
    return full


# revision 40
# speedup vs baseline: 4.7333x; 1.0087x over previous
"""DCGRU cell on 8 Trainium2 NeuronCores.

Sharding: data-parallel over batch (B=32 -> 4 per core), adjacency + MLP
weights replicated. No collectives; host gathers per-core outputs.

Key structure (all matmuls fp8 e4m3, DoubleRow perf mode = 2 contraction
subtiles per instruction):
  - W_fwd/W_bwd cached in SBUF as WT fp8 tiles [128, 16, 2048] scaled
    x1024 (row-stochastic entries ~1e-3 are subnormal in e4m3 otherwise).
  - Diffusion 1 (r,z gates): feature-major hop outputs fm2 [128, 6, N]
    with planes [h_b0 | x_b0;x_b1 | h_b1 | h_b2 | x_b2;x_b3 | h_b3]; batch
    b's 192 features = plane pair (pb, pb+1), pb = [0,1,3,4][b]. MLP feed
    = ONE DoubleRow matmul per (gate, chunk, b, nblk) with host-packed
    weight variants (x-rows zero-padded to match the shared x-pair
    plane). Logits accumulate in SBUF bf16 at 32x scale; sigmoid applies
    scale=2^-5. Chain re-entry fm->nm via PE transposes (hops 1,2).
  - Diffusion 2 (n gate): Horner chain T = W(y1 + W(y2 + W y3)) over
    node-major state t [128, 16, 512] (cols = b*128+o). Projections
    y_k = Wn_k^T x_rh are extra matmuls accumulated into the same PSUM
    group (x_rh feature-major as stationary operand). No transposes, no
    DRAM spill. n-logits land node-major, transposed once at the end.
  - Scale chain: hop-1 stores x2^-7 (=8x true), hops 2-3 x2^-10 (=8x),
    d1 MLP weights x32 (chunk 0) / x4 (chunks 1-6); d2 weights x1024,
    chain copies 2^-10, final tanh scale 2^-10.
  - ~9 DMAs total per core (DMA dispatch, not bandwidth, dominated the
    old design).
"""

import sys
import numpy as np
import ml_dtypes

for _p in ("/opt/trn_rl_repo",):
    if _p not in sys.path:
        sys.path.insert(0, _p)

from concourse import bacc, tile, mybir  # noqa: E402
from concourse.bass_utils import run_bass_kernel_spmd  # noqa: E402

F32 = mybir.dt.float32
BF16 = mybir.dt.bfloat16
FP8 = mybir.dt.float8e4
AF = mybir.ActivationFunctionType
DR = mybir.MatmulPerfMode.DoubleRow
NP_FP8 = ml_dtypes.float8_e4m3

C = 4          # batches per core
FI = 192       # per-batch feature width (x 64 + h 128)
BF = C * FI    # 768
DH = 128
NCORES = 8
NHOPS = 3

W2P = [0, 2, 3, 5, 1, 4]   # hop psum window -> fm2 plane
PB = [0, 1, 3, 4]          # batch -> first fm2 plane of its (lo, hi) pair

WSCALE = 1024.0            # W_fwd/W_bwd host prescale
HOP1_SCALE = 2.0 ** -7     # psum -> fm2 store, hop 1 (keeps feats at 8x)
HOPK_SCALE = 2.0 ** -10    # psum -> fm2 store, hops 2+
GATE_SCALE = 2.0 ** -5     # r/z logits accumulate at 32x
CHAIN_SCALE = 2.0 ** -10   # d2 chain psum -> t store / final tanh


def build_nc(nt=16):
    """Build + compile the per-core Bass kernel. nt = node tiles (N = nt*128)."""
    N = nt * 128
    npair = nt // 2

    nc = bacc.Bacc("TRN2", target_bir_lowering=False, debug=False,
                   num_devices=NCORES)

    def din(name, shape, dt=FP8):
        return nc.dram_tensor(name, shape, dt, kind="ExternalInput").ap()

    WF = din("wfT", [128, nt, N])
    WB = din("wbT", [128, nt, N])
    XNM = din("x_nm", [128, nt, BF])
    FM0 = din("fm0", [128, 6, N])
    HB = din("h_fm", [128, C, N], BF16)
    WX1 = din("wx1", [128, 56, 128])       # ((g*7+k)*2+v)*2 -> 2 planes
    WX2 = din("wx2", [128, 32, 128])       # (k*2+v)*2 -> 2 planes; 28-31 dW0 res
    WXB = din("wxb", [128, 2, 128], BF16)  # 32*Wr0_h.T | 32*Wz0_h.T
    IDT = din("ident", [128, 128])
    IDTB = din("identb", [128, 128], BF16)
    BIAS = din("bias", [128, 4], F32)      # br | bz | bn | -bz
    OUT = nc.dram_tensor("out_fm", [C, 128, N], F32, kind="ExternalOutput").ap()

    with tile.TileContext(nc) as tc:
        with (
            tc.tile_pool(name="w", bufs=4) as w_pool,
            tc.tile_pool(name="xnm", bufs=2) as xnm_pool,
            tc.tile_pool(name="fm", bufs=3) as fm_pool,
            tc.tile_pool(name="fm0", bufs=1) as fm0_pool,
            tc.tile_pool(name="acc", bufs=2) as acc_pool,
            tc.tile_pool(name="h", bufs=1) as h_pool,
            tc.tile_pool(name="wx", bufs=1) as wx_pool,
            tc.tile_pool(name="gate", bufs=1) as gate_pool,
            tc.tile_pool(name="sg", bufs=3) as sg_pool,
            tc.tile_pool(name="const", bufs=1) as const_pool,
            tc.tile_pool(name="psh", bufs=4, space="PSUM") as psh_pool,
            tc.tile_pool(name="psm", bufs=2, space="PSUM") as psm_pool,
            tc.tile_pool(name="pst", bufs=2, space="PSUM") as pst_pool,
        ):
            # ---- one-time loads (ordered so PE can start ASAP) ----
            wxb = const_pool.tile([128, 2, 128], BF16, name="wxb_t", tag="wxb")
            nc.sync.dma_start(wxb[:], WXB[:])
            hb = h_pool.tile([128, C, N], BF16, name="h_t", tag="h")
            nc.sync.dma_start(hb[:], HB[:])
            fm0 = fm0_pool.tile([128, 6, N], FP8, name="fm0_t", tag="fm0")
            nc.sync.dma_start(fm0[:], FM0[:])
            wx1 = wx_pool.tile([128, 56 + 32, 128], FP8, name="wx_t", tag="wx")
            nc.sync.dma_start(wx1[:, 0:56, :], WX1[:])
            nc.sync.dma_start(wx1[:, 56:88, :], WX2[:])
            cur = xnm_pool.tile([128, nt, BF], FP8, name="xnm_t", tag="xnm")
            nc.sync.dma_start(cur[:], XNM[:])
            hnt = nt // 2
            wf = [w_pool.tile([128, hnt, N], FP8, name="wt_t", tag="w")
                  for _ in range(2)]
            wb = [w_pool.tile([128, hnt, N], FP8, name="wt_t", tag="w")
                  for _ in range(2)]
            for hx in range(2):
                nc.sync.dma_start(wf[hx][:], WF[:, hnt * hx:hnt * (hx + 1), :])
            for hx in range(2):
                nc.sync.dma_start(wb[hx][:], WB[:, hnt * hx:hnt * (hx + 1), :])
            ident = const_pool.tile([128, 128], FP8, name="ident_t", tag="ident")
            nc.sync.dma_start(ident[:], IDT[:])
            identb = const_pool.tile([128, 128], BF16, name="identb_t", tag="identb")
            nc.sync.dma_start(identb[:], IDTB[:])
            bias = const_pool.tile([128, 4], F32, name="bias_t", tag="bias")
            nc.sync.dma_start(bias[:], BIAS[:])

            accr = acc_pool.tile([128, C, N], BF16, name="acc_t", tag="acc")
            accz = acc_pool.tile([128, C, N], BF16, name="acc_t", tag="acc")

            def hop_mm(ps, wt2, xnm2, ccols, ibs):
                """Full-contraction DR group: out[ccols-window, ibs]."""
                for jp in range(npair):
                    h2, j2 = divmod(2 * jp, hnt)
                    nc.tensor.matmul(
                        ps[:], xnm2[:, 2 * jp:2 * jp + 2, ccols],
                        wt2[h2][:, j2:j2 + 2, ibs],
                        start=(jp == 0), stop=(jp == npair - 1),
                        perf_mode=DR)

            def mlp_feed_b(srcs, b, g, acc):
                for ib in range(4):
                    nbs = slice(512 * ib, 512 * (ib + 1))
                    ps = psm_pool.tile([128, 512], F32, name="psm_t", tag="psm")
                    for s, (src, kidx) in enumerate(srcs):
                        widx = ((g * 7 + kidx) * 2 + (b & 1)) * 2
                        nc.tensor.matmul(
                            ps[:], wx1[:, widx:widx + 2, :],
                            src[:, PB[b]:PB[b] + 2, nbs],
                            start=(s == 0), stop=(s == len(srcs) - 1),
                            perf_mode=DR)
                    nc.vector.tensor_add(acc[:, b, nbs], ps[:],
                                         acc[:, b, nbs])

            def mlp_feed(srcs, init, gates, addeng=None):
                """srcs: list of (fm2, kidx) chunk pairs in one psum group.
                Accumulate bf16 logits; adds alternate DVE / Pool engines."""
                for g, acc in gates:
                    for b in range(C):
                        for ib in range(4):
                            nbs = slice(512 * ib, 512 * (ib + 1))
                            ps = psm_pool.tile([128, 512], F32, name="psm_t", tag="psm")
                            for s, (src, kidx) in enumerate(srcs):
                                widx = ((g * 7 + kidx) * 2 + (b & 1)) * 2
                                nc.tensor.matmul(
                                    ps[:], wx1[:, widx:widx + 2, :],
                                    src[:, PB[b]:PB[b] + 2, nbs],
                                    start=(s == 0), stop=(s == len(srcs) - 1),
                                    perf_mode=DR)
                            if init:
                                nc.vector.tensor_copy(acc[:, b, nbs], ps[:])
                            else:
                                nc.vector.tensor_add(acc[:, b, nbs], ps[:],
                                                     acc[:, b, nbs])

            # ---------------- diffusion 1 (r, z gates) ----------------
            # chunk-0 feed: h-part in bf16 (hb, wxb), x-part fp8 single plane
            for g, acc in ((0, accr), (1, accz)):
                for b in range(C):
                    widx = (g * 7 * 2 + (b & 1)) * 2
                    xw = widx + (0 if b & 1 else 1)
                    xpl = PB[b] if b & 1 else PB[b] + 1
                    for ib in range(4):
                        nbs = slice(512 * ib, 512 * (ib + 1))
                        ps = psm_pool.tile([128, 512], F32, name="psm_t", tag="psm")
                        nc.tensor.matmul(
                            ps[:], wxb[:, g, :], hb[:, b, nbs],
                            start=True, stop=False, skip_group_check=True)
                        nc.tensor.matmul(
                            ps[:], wx1[:, xw, :], fm0[:, xpl, nbs],
                            start=False, stop=True, skip_group_check=True)
                        nc.vector.tensor_copy(acc[:, b, nbs], ps[:])

            fm_hist = {}
            for wdir, wt2 in ((0, wf), (1, wb)):
                if wdir == 1:
                    cur = xnm_pool.tile([128, nt, BF], FP8, name="xnm_t", tag="xnm")
                    nc.sync.dma_start(cur[:], XNM[:])
                for k in range(1, NHOPS + 1):
                    cps = HOP1_SCALE if k == 1 else HOPK_SCALE
                    kidx = wdir * NHOPS + k
                    fm2 = fm_pool.tile([128, 6, N], FP8, name="fm_t", tag="fm")
                    fm_hist[kidx] = fm2
                    nxt = (xnm_pool.tile([128, nt, BF], FP8, name="xnm_t", tag="xnm")
                           if k < NHOPS else None)
                    def emit_tr(c, ib):
                        # fm -> nm re-entry (it-blocks 4*ib..4*ib+3):
                        # transpose via plain fp8 matmul against the identity
                        # (fp8 transpose mode is rejected by the compiler)
                        ccols = slice(128 * c, 128 * (c + 1))
                        pt = pst_pool.tile([128, 4, 128], F32,
                                           name="pst_t", tag="pst")
                        for i in range(4):
                            it = 4 * ib + i
                            nc.tensor.matmul(
                                pt[:, i, :],
                                fm2[:, W2P[c], 128 * it:128 * (it + 1)],
                                ident[:], start=True, stop=True)
                        nc.scalar.activation(
                            nxt[:, 4 * ib:4 * ib + 4, ccols], pt[:], AF.Copy)

                    pend = None
                    for c in range(6):
                        ccols = slice(128 * c, 128 * (c + 1))
                        for ib in range(4):
                            ibs = slice(512 * ib, 512 * (ib + 1))
                            ps = psh_pool.tile([128, 512], F32, name="psh_t", tag="psh")
                            hop_mm(ps, wt2, cur, ccols, ibs)
                            last = (wdir, k) == (1, NHOPS)
                            eng = (nc.vector if last or (c * 4 + ib) & 1
                                   else nc.scalar)
                            if eng is nc.vector:
                                nc.vector.tensor_scalar_mul(
                                    fm2[:, W2P[c], ibs], ps[:], cps)
                            else:
                                nc.scalar.activation(
                                    fm2[:, W2P[c], ibs], ps[:], AF.Copy,
                                    scale=cps)
                            if nxt is not None:
                                if pend is not None:
                                    emit_tr(*pend)
                                pend = (c, ib)
                    if pend is not None:
                        emit_tr(*pend)
                    # feed chunk triples {1,2,3} / {4,5,6} once all exist
                    fk = {(0, 3): (1, 2, 3), (1, 3): (4, 5, 6)}.get((wdir, k))
                    if fk == (1, 2, 3):
                        mlp_feed([(fm_hist[j], j) for j in fk], False,
                                 [(0, accr), (1, accz)])
                    elif fk is not None:
                        # last round: per-batch r adds -> sigmoid -> r*h so
                        # the d2 chains can start before the z adds finish
                        srcs = [(fm_hist[j], j) for j in fk]
                        for b in range(C):
                            mlp_feed_b(srcs, b, 0, accr)
                            r = gate_pool.tile([128, N], BF16, name="gate_t",
                                               tag="gate")
                            nc.scalar.activation(r[:], accr[:, b, :],
                                                 AF.Sigmoid,
                                                 bias=bias[:, 0:1],
                                                 scale=GATE_SCALE)
                            nc.vector.tensor_mul(fm0[:, W2P[b], :], r[:],
                                                 hb[:, b, :])
                        for b in range(C):
                            mlp_feed_b(srcs, b, 1, accz)
                    if nxt is not None:
                        cur = nxt

            xrh2 = fm0

            # ---- z and u = (1-z)*h, computed while the chains run ----
            uh = [fm_pool.tile([128, 2, N], BF16, name="fm_t", tag="fm")
                  for _ in range(2)]
            for b in range(C):
                zc = gate_pool.tile([128, N], BF16, name="gate_t", tag="gate")
                nc.scalar.activation(zc[:], accz[:, b, :], AF.Sigmoid,
                                     bias=bias[:, 3:4], scale=-GATE_SCALE)
                nc.gpsimd.tensor_mul(uh[b // 2][:, b & 1, :], zc[:],
                                     hb[:, b, :])
            for b in range(C):
                nc.scalar.activation(accz[:, b, :], accz[:, b, :], AF.Sigmoid,
                                     bias=bias[:, 1:2], scale=GATE_SCALE)

            # ---------------- diffusion 2 (n gate): Horner chains ----------------
            accn = acc_pool.tile([128, nt, 512], BF16, name="acc_t", tag="acc")

            def proj_it(ps, it, kidx, start, stop):
                """y_k projection matmuls for one node block into psum ps."""
                for b in range(C):
                    widx = 56 + (kidx * 2 + (b & 1)) * 2
                    nc.tensor.matmul(
                        ps[:, 128 * b:128 * (b + 1)],
                        xrh2[:, PB[b]:PB[b] + 2, 128 * it:128 * (it + 1)],
                        wx1[:, widx:widx + 2, :],
                        start=start, stop=stop, perf_mode=DR,
                        skip_group_check=True)

            # y0 -> accn; extra DR matmul adds the fp8 weight residual
            for it in range(nt):
                ps = psh_pool.tile([128, 512], F32, name="psh_t", tag="psh")
                for b in range(C):
                    widx = 56 + (b & 1) * 2
                    rwidx = 84 + (b & 1) * 2
                    xs = xrh2[:, PB[b]:PB[b] + 2, 128 * it:128 * (it + 1)]
                    nc.tensor.matmul(
                        ps[:, 128 * b:128 * (b + 1)], xs, wx1[:, widx:widx + 2, :],
                        start=True, stop=False, perf_mode=DR,
                        skip_group_check=True)
                    nc.tensor.matmul(
                        ps[:, 128 * b:128 * (b + 1)], xs, wx1[:, rwidx:rwidx + 2, :],
                        start=False, stop=True, perf_mode=DR,
                        skip_group_check=True)
                nc.vector.tensor_copy(accn[:, it, :], ps[:])

            def final_gate_q(q):
                """out = u + z*tanh(n) for node block q (all batches)."""
                qs = slice(512 * q, 512 * (q + 1))
                for b in range(C):
                    pt = pst_pool.tile([128, 512], BF16, name="pst_t", tag="pst")
                    for i in range(4):
                        it = 4 * q + i
                        nc.tensor.transpose(
                            pt[:, 128 * i:128 * (i + 1)],
                            accn[:, it, 128 * b:128 * (b + 1)], identb[:])
                    n_t = sg_pool.tile([128, 512], F32, name="sg_t", tag="sg")
                    nc.scalar.activation(n_t[:], pt[:], AF.Tanh,
                                         bias=bias[:, 2:3], scale=CHAIN_SCALE)
                    zn = sg_pool.tile([128, 512], F32, name="sg_t", tag="sg")
                    nc.gpsimd.tensor_mul(zn[:], n_t[:], accz[:, b, qs])
                    og = sg_pool.tile([128, 512], F32, name="og_t", tag="sg")
                    nc.gpsimd.tensor_add(og[:], zn[:], uh[b // 2][:, b & 1, qs])
                    nc.sync.dma_start(OUT[b][:, qs], og[:])

            for wdir, wt2 in ((0, wf), (1, wb)):
                # t = y_3 (projection only)
                t = xnm_pool.tile([128, nt, 512], FP8, name="tch_t", tag="xnm")
                for it in range(nt):
                    ps = psh_pool.tile([128, 512], F32, name="psh_t", tag="psh")
                    proj_it(ps, it, wdir * NHOPS + NHOPS, True, True)
                    nc.vector.tensor_scalar_mul(t[:, it, :], ps[:], CHAIN_SCALE)
                for k in range(NHOPS, 0, -1):
                    # next state = W*t (+ y_{k-1} while k>1)
                    tn = (xnm_pool.tile([128, nt, 512], FP8, name="tch_t", tag="xnm")
                          if k > 1 else None)
                    for it in range(nt):
                        ccols = slice(128 * it, 128 * (it + 1))
                        ps = psh_pool.tile([128, 512], F32, name="psh_t", tag="psh")
                        for jp in range(npair):
                            h2, j2 = divmod(2 * jp, hnt)
                            nc.tensor.matmul(
                                ps[:], wt2[h2][:, j2:j2 + 2, ccols],
                                t[:, 2 * jp:2 * jp + 2, :],
                                start=(jp == 0), stop=(jp == npair - 1 and k == 1),
                                perf_mode=DR, skip_group_check=True)
                        if k > 1:
                            proj_it(ps, it, wdir * NHOPS + k - 1, False, True)
                            if it & 1:
                                nc.scalar.activation(tn[:, it, :], ps[:],
                                                     AF.Copy, scale=CHAIN_SCALE)
                            else:
                                nc.vector.tensor_scalar_mul(
                                    tn[:, it, :], ps[:], CHAIN_SCALE)
                        else:
                            nc.vector.tensor_add(accn[:, it, :], ps[:],
                                                 accn[:, it, :])
                            if wdir == 1 and it % 4 == 3:
                                final_gate_q(it // 4)
                    if k > 1:
                        t = tn

    nc.compile()
    return nc


def _pack_gate_variants(W, scales):
    """Torch-Linear weight [128, 7*192] -> [7*2, 2, 64... ] DR variants.

    For chunk k (feature slice [k*192:(k+1)*192] = [x(64) | h(128)]) emit
    variant A (even b: planes (lo=h, hi=x-upper)) and B (odd b: planes
    (x-lower, h)), each [2, 128, 128] with rows = contraction features and
    cols = output unit. Returns [7, 2, 2, 128, 128] float32.
    """
    out = np.zeros((7, 2, 2, 128, 128), np.float32)
    for k in range(7):
        s = scales[k]
        Wx = s * W[:, k * FI:k * FI + 64].T          # [64, 128]
        Wh = s * W[:, k * FI + 64:(k + 1) * FI].T    # [128, 128]
        out[k, 0, 0] = Wh
        out[k, 0, 1, 0:64] = Wx
        out[k, 1, 0, 64:128] = Wx
        out[k, 1, 1] = Wh
    return out


_NC_CACHE = {}


def _get_nc(nt):
    if nt not in _NC_CACHE:
        _NC_CACHE[nt] = build_nc(nt)
    return _NC_CACHE[nt]


def make_in_maps(x, h_prev, W_fwd, W_bwd, Wr, br, Wz, bz, Wn, bn):
    x = np.asarray(x, np.float32)
    h_prev = np.asarray(h_prev, np.float32)
    B, N, Din = x.shape
    nt = N // 128

    def to_pmajor(a):
        # [N(j), cols] -> [128(p), nt(jt), cols] with j = jt*128 + p
        cols = a.shape[1]
        return np.ascontiguousarray(
            a.reshape(nt, 128, cols).transpose(1, 0, 2))

    WfT = to_pmajor(np.asarray(W_fwd, np.float32).T * WSCALE).astype(NP_FP8)
    WbT = to_pmajor(np.asarray(W_bwd, np.float32).T * WSCALE).astype(NP_FP8)

    d1scales = [32.0] + [4.0] * 6
    wr_v = _pack_gate_variants(np.asarray(Wr, np.float32), d1scales)
    wz_v = _pack_gate_variants(np.asarray(Wz, np.float32), d1scales)
    wn_v = _pack_gate_variants(np.asarray(Wn, np.float32), [WSCALE] * 7)
    wn8 = wn_v.astype(NP_FP8).astype(np.float32)
    dwn0 = wn_v[0] - wn8[0]                      # chunk-0 weight fp8 residual
    # wx1 [128, 56, 128]: plane ((g*7+k)*2+v)*2 + pl
    wx1 = np.concatenate([wr_v, wz_v]).reshape(14, 2, 2, 128, 128)
    wx1 = np.ascontiguousarray(
        wx1.reshape(28, 2, 128, 128).reshape(56, 128, 128)
        .transpose(1, 0, 2)).astype(NP_FP8)
    wx2 = np.concatenate(
        [wn_v.reshape(28, 128, 128), dwn0.reshape(4, 128, 128)])
    wx2 = np.ascontiguousarray(wx2.transpose(1, 0, 2)).astype(NP_FP8)
    wxb = np.stack([32.0 * np.asarray(Wr, np.float32)[:, 64:192].T,
                    32.0 * np.asarray(Wz, np.float32)[:, 64:192].T])
    wxb = np.ascontiguousarray(
        wxb.transpose(1, 0, 2)).astype(ml_dtypes.bfloat16)

    ident8 = np.eye(128, dtype=np.float32).astype(NP_FP8)
    identb = np.eye(128, dtype=np.float32).astype(ml_dtypes.bfloat16)
    biases = np.stack([np.asarray(br, np.float32),
                       np.asarray(bz, np.float32),
                       np.asarray(bn, np.float32),
                       -np.asarray(bz, np.float32)], axis=1)  # [128, 4]
    biases = np.ascontiguousarray(biases)

    ncores = B // C
    in_maps = []
    for cix in range(ncores):
        xs = x[C * cix:C * (cix + 1)]            # [C, N, 64]
        hs = h_prev[C * cix:C * (cix + 1)]       # [C, N, 128]
        # x_nm cols: [h_b0|h_b1|h_b2|h_b3|x_b0|x_b1|x_b2|x_b3]
        xnm_cols = np.concatenate(
            [hs[b] for b in range(C)] + [xs[b] for b in range(C)], axis=1)
        x_nm = to_pmajor(xnm_cols).astype(NP_FP8)
        # fm0 planes: [h0.T | x0.T;x1.T | h1.T | h2.T | x2.T;x3.T | h3.T]
        hT = [np.ascontiguousarray(hs[b].T) for b in range(C)]
        xT = [np.ascontiguousarray(xs[b].T) for b in range(C)]
        fm0 = np.stack([
            hT[0], np.concatenate([xT[0], xT[1]], axis=0), hT[1],
            hT[2], np.concatenate([xT[2], xT[3]], axis=0), hT[3],
        ], axis=1)                               # [128, 6, N]
        h_fm = np.ascontiguousarray(
            np.stack(hT, axis=1)).astype(ml_dtypes.bfloat16)  # [128, C, N]
        in_maps.append(dict(
            wfT=WfT, wbT=WbT, x_nm=x_nm,
            fm0=np.ascontiguousarray(fm0).astype(NP_FP8),
            h_fm=h_fm, wx1=wx1, wx2=wx2, wxb=wxb,
            ident=ident8, identb=identb, bias=biases))
    return in_maps, nt, ncores


def kernel(x, h_prev, W_fwd, W_bwd, Wr, br, Wz, bz, Wn, bn, _trace=False):
    in_maps, nt, ncores = make_in_maps(
        x, h_prev, W_fwd, W_bwd, Wr, br, Wz, bz, Wn, bn)
    nc = _get_nc(nt)
    res = run_bass_kernel_spmd(nc, in_maps, list(range(ncores)), trace=_trace)
    outs = [np.ascontiguousarray(res.results[c]["out_fm"].transpose(0, 2, 1))
            for c in range(ncores)]
    full = np.concatenate(outs, axis=0).astype(np.float32)
    if _trace:
        return full, res
    return full


# revision 43
# speedup vs baseline: 4.7339x; 1.0001x over previous
"""DCGRU cell on 8 Trainium2 NeuronCores.

Sharding: data-parallel over batch (B=32 -> 4 per core), adjacency + MLP
weights replicated. No collectives; host gathers per-core outputs.

Key structure (all matmuls fp8 e4m3, DoubleRow perf mode = 2 contraction
subtiles per instruction):
  - W_fwd/W_bwd cached in SBUF as WT fp8 tiles [128, 16, 2048] scaled
    x1024 (row-stochastic entries ~1e-3 are subnormal in e4m3 otherwise).
  - Diffusion 1 (r,z gates): feature-major hop outputs fm2 [128, 6, N]
    with planes [h_b0 | x_b0;x_b1 | h_b1 | h_b2 | x_b2;x_b3 | h_b3]; batch
    b's 192 features = plane pair (pb, pb+1), pb = [0,1,3,4][b]. MLP feed
    = ONE DoubleRow matmul per (gate, chunk, b, nblk) with host-packed
    weight variants (x-rows zero-padded to match the shared x-pair
    plane). Logits accumulate in SBUF bf16 at 32x scale; sigmoid applies
    scale=2^-5. Chain re-entry fm->nm via PE transposes (hops 1,2).
  - Diffusion 2 (n gate): Horner chain T = W(y1 + W(y2 + W y3)) over
    node-major state t [128, 16, 512] (cols = b*128+o). Projections
    y_k = Wn_k^T x_rh are extra matmuls accumulated into the same PSUM
    group (x_rh feature-major as stationary operand). No transposes, no
    DRAM spill. n-logits land node-major, transposed once at the end.
  - Scale chain: hop-1 stores x2^-7 (=8x true), hops 2-3 x2^-10 (=8x),
    d1 MLP weights x32 (chunk 0) / x4 (chunks 1-6); d2 weights x1024,
    chain copies 2^-10, final tanh scale 2^-10.
  - ~9 DMAs total per core (DMA dispatch, not bandwidth, dominated the
    old design).
"""

import sys
import numpy as np
import ml_dtypes

for _p in ("/opt/trn_rl_repo",):
    if _p not in sys.path:
        sys.path.insert(0, _p)

from concourse import bacc, tile, mybir  # noqa: E402
from concourse.bass_utils import run_bass_kernel_spmd  # noqa: E402

F32 = mybir.dt.float32
BF16 = mybir.dt.bfloat16
FP8 = mybir.dt.float8e4
AF = mybir.ActivationFunctionType
DR = mybir.MatmulPerfMode.DoubleRow
NP_FP8 = ml_dtypes.float8_e4m3

C = 4          # batches per core
FI = 192       # per-batch feature width (x 64 + h 128)
BF = C * FI    # 768
DH = 128
NCORES = 8
NHOPS = 3

W2P = [0, 2, 3, 5, 1, 4]   # hop psum window -> fm2 plane
PB = [0, 1, 3, 4]          # batch -> first fm2 plane of its (lo, hi) pair

WSCALE = 1024.0            # W_fwd/W_bwd host prescale
HOP1_SCALE = 2.0 ** -7     # psum -> fm2 store, hop 1 (keeps feats at 8x)
HOPK_SCALE = 2.0 ** -10    # psum -> fm2 store, hops 2+
GATE_SCALE = 2.0 ** -5     # r/z logits accumulate at 32x
CHAIN_SCALE = 2.0 ** -10   # d2 chain psum -> t store / final tanh


def build_nc(nt=16):
    """Build + compile the per-core Bass kernel. nt = node tiles (N = nt*128)."""
    N = nt * 128
    npair = nt // 2

    nc = bacc.Bacc("TRN2", target_bir_lowering=False, debug=False,
                   num_devices=NCORES)

    def din(name, shape, dt=FP8):
        return nc.dram_tensor(name, shape, dt, kind="ExternalInput").ap()

    WF = din("wfT", [128, nt, N])
    WB = din("wbT", [128, nt, N])
    XNM = din("x_nm", [128, nt, BF])
    FM0 = din("fm0", [128, 6, N])
    HB = din("h_fm", [128, C, N], BF16)
    WX1 = din("wx1", [128, 56, 128])       # ((g*7+k)*2+v)*2 -> 2 planes
    WX2 = din("wx2", [128, 32, 128])       # (k*2+v)*2 -> 2 planes; 28-31 dW0 res
    WXB = din("wxb", [128, 2, 128], BF16)  # 32*Wr0_h.T | 32*Wz0_h.T
    IDT = din("ident", [128, 128])
    IDTB = din("identb", [128, 128], BF16)
    BIAS = din("bias", [128, 4], F32)      # br | bz | bn | -bz
    OUT = nc.dram_tensor("out_fm", [C, 128, N], F32, kind="ExternalOutput").ap()

    with tile.TileContext(nc) as tc:
        with (
            tc.tile_pool(name="w", bufs=4) as w_pool,
            tc.tile_pool(name="xnm", bufs=2) as xnm_pool,
            tc.tile_pool(name="fm", bufs=3) as fm_pool,
            tc.tile_pool(name="fm0", bufs=1) as fm0_pool,
            tc.tile_pool(name="acc", bufs=2) as acc_pool,
            tc.tile_pool(name="h", bufs=1) as h_pool,
            tc.tile_pool(name="wx", bufs=1) as wx_pool,
            tc.tile_pool(name="gate", bufs=1) as gate_pool,
            tc.tile_pool(name="sg", bufs=3) as sg_pool,
            tc.tile_pool(name="const", bufs=1) as const_pool,
            tc.tile_pool(name="psh", bufs=4, space="PSUM") as psh_pool,
            tc.tile_pool(name="psm", bufs=2, space="PSUM") as psm_pool,
            tc.tile_pool(name="pst", bufs=2, space="PSUM") as pst_pool,
        ):
            # ---- one-time loads (ordered so PE can start ASAP) ----
            wxb = const_pool.tile([128, 2, 128], BF16, name="wxb_t", tag="wxb")
            nc.sync.dma_start(wxb[:], WXB[:])
            hb = h_pool.tile([128, C, N], BF16, name="h_t", tag="h")
            nc.sync.dma_start(hb[:, 0:2, :], HB[:, 0:2, :])
            nc.sync.dma_start(hb[:, 2:4, :], HB[:, 2:4, :])
            fm0 = fm0_pool.tile([128, 6, N], FP8, name="fm0_t", tag="fm0")
            nc.sync.dma_start(fm0[:], FM0[:])
            wx1 = wx_pool.tile([128, 56 + 32, 128], FP8, name="wx_t", tag="wx")
            nc.sync.dma_start(wx1[:, 0:56, :], WX1[:])
            nc.sync.dma_start(wx1[:, 56:88, :], WX2[:])
            cur = xnm_pool.tile([128, nt, BF], FP8, name="xnm_t", tag="xnm")
            nc.sync.dma_start(cur[:], XNM[:])
            hnt = nt // 2
            wf = [w_pool.tile([128, hnt, N], FP8, name="wt_t", tag="w")
                  for _ in range(2)]
            wb = [w_pool.tile([128, hnt, N], FP8, name="wt_t", tag="w")
                  for _ in range(2)]
            for hx in range(2):
                nc.sync.dma_start(wf[hx][:], WF[:, hnt * hx:hnt * (hx + 1), :])
            for hx in range(2):
                nc.sync.dma_start(wb[hx][:], WB[:, hnt * hx:hnt * (hx + 1), :])
            ident = const_pool.tile([128, 128], FP8, name="ident_t", tag="ident")
            nc.sync.dma_start(ident[:], IDT[:])
            identb = const_pool.tile([128, 128], BF16, name="identb_t", tag="identb")
            nc.sync.dma_start(identb[:], IDTB[:])
            bias = const_pool.tile([128, 4], F32, name="bias_t", tag="bias")
            nc.sync.dma_start(bias[:], BIAS[:])

            accr = acc_pool.tile([128, C, N], BF16, name="acc_t", tag="acc")
            accz = acc_pool.tile([128, C, N], BF16, name="acc_t", tag="acc")

            def hop_mm(ps, wt2, xnm2, ccols, ibs):
                """Full-contraction DR group: out[ccols-window, ibs]."""
                for jp in range(npair):
                    h2, j2 = divmod(2 * jp, hnt)
                    nc.tensor.matmul(
                        ps[:], xnm2[:, 2 * jp:2 * jp + 2, ccols],
                        wt2[h2][:, j2:j2 + 2, ibs],
                        start=(jp == 0), stop=(jp == npair - 1),
                        perf_mode=DR)

            def mlp_feed_b(srcs, b, g, acc):
                for ib in range(4):
                    nbs = slice(512 * ib, 512 * (ib + 1))
                    ps = psm_pool.tile([128, 512], F32, name="psm_t", tag="psm")
                    for s, (src, kidx) in enumerate(srcs):
                        widx = ((g * 7 + kidx) * 2 + (b & 1)) * 2
                        nc.tensor.matmul(
                            ps[:], wx1[:, widx:widx + 2, :],
                            src[:, PB[b]:PB[b] + 2, nbs],
                            start=(s == 0), stop=(s == len(srcs) - 1),
                            perf_mode=DR)
                    nc.vector.tensor_add(acc[:, b, nbs], ps[:],
                                         acc[:, b, nbs])

            def mlp_feed(srcs, init, gates, addeng=None):
                """srcs: list of (fm2, kidx) chunk pairs in one psum group.
                Accumulate bf16 logits; adds alternate DVE / Pool engines."""
                for g, acc in gates:
                    for b in range(C):
                        for ib in range(4):
                            nbs = slice(512 * ib, 512 * (ib + 1))
                            ps = psm_pool.tile([128, 512], F32, name="psm_t", tag="psm")
                            for s, (src, kidx) in enumerate(srcs):
                                widx = ((g * 7 + kidx) * 2 + (b & 1)) * 2
                                nc.tensor.matmul(
                                    ps[:], wx1[:, widx:widx + 2, :],
                                    src[:, PB[b]:PB[b] + 2, nbs],
                                    start=(s == 0), stop=(s == len(srcs) - 1),
                                    perf_mode=DR)
                            if init:
                                nc.vector.tensor_copy(acc[:, b, nbs], ps[:])
                            else:
                                nc.vector.tensor_add(acc[:, b, nbs], ps[:],
                                                     acc[:, b, nbs])

            # ---------------- diffusion 1 (r, z gates) ----------------
            # chunk-0 feed: h-part in bf16 (hb, wxb), x-part fp8 single plane
            for g, acc in ((0, accr), (1, accz)):
                for b in range(C):
                    widx = (g * 7 * 2 + (b & 1)) * 2
                    xw = widx + (0 if b & 1 else 1)
                    xpl = PB[b] if b & 1 else PB[b] + 1
                    for ib in range(4):
                        nbs = slice(512 * ib, 512 * (ib + 1))
                        ps = psm_pool.tile([128, 512], F32, name="psm_t", tag="psm")
                        nc.tensor.matmul(
                            ps[:], wxb[:, g, :], hb[:, b, nbs],
                            start=True, stop=False, skip_group_check=True)
                        nc.tensor.matmul(
                            ps[:], wx1[:, xw, :], fm0[:, xpl, nbs],
                            start=False, stop=True, skip_group_check=True)
                        nc.vector.tensor_copy(acc[:, b, nbs], ps[:])

            fm_hist = {}
            for wdir, wt2 in ((0, wf), (1, wb)):
                if wdir == 1:
                    cur = xnm_pool.tile([128, nt, BF], FP8, name="xnm_t", tag="xnm")
                    nc.sync.dma_start(cur[:], XNM[:])
                for k in range(1, NHOPS + 1):
                    cps = HOP1_SCALE if k == 1 else HOPK_SCALE
                    kidx = wdir * NHOPS + k
                    fm2 = fm_pool.tile([128, 6, N], FP8, name="fm_t", tag="fm")
                    fm_hist[kidx] = fm2
                    nxt = (xnm_pool.tile([128, nt, BF], FP8, name="xnm_t", tag="xnm")
                           if k < NHOPS else None)
                    def emit_tr(c, ib):
                        # fm -> nm re-entry (it-blocks 4*ib..4*ib+3):
                        # transpose via plain fp8 matmul against the identity
                        # (fp8 transpose mode is rejected by the compiler)
                        ccols = slice(128 * c, 128 * (c + 1))
                        pt = pst_pool.tile([128, 4, 128], F32,
                                           name="pst_t", tag="pst")
                        for i in range(4):
                            it = 4 * ib + i
                            nc.tensor.matmul(
                                pt[:, i, :],
                                fm2[:, W2P[c], 128 * it:128 * (it + 1)],
                                ident[:], start=True, stop=True)
                        nc.scalar.activation(
                            nxt[:, 4 * ib:4 * ib + 4, ccols], pt[:], AF.Copy)

                    pend = None
                    for c in range(6):
                        ccols = slice(128 * c, 128 * (c + 1))
                        for ib in range(4):
                            ibs = slice(512 * ib, 512 * (ib + 1))
                            ps = psh_pool.tile([128, 512], F32, name="psh_t", tag="psh")
                            hop_mm(ps, wt2, cur, ccols, ibs)
                            last = (wdir, k) == (1, NHOPS)
                            eng = (nc.vector if last or (c * 4 + ib) & 1
                                   else nc.scalar)
                            if eng is nc.vector:
                                nc.vector.tensor_scalar_mul(
                                    fm2[:, W2P[c], ibs], ps[:], cps)
                            else:
                                nc.scalar.activation(
                                    fm2[:, W2P[c], ibs], ps[:], AF.Copy,
                                    scale=cps)
                            if nxt is not None:
                                if pend is not None:
                                    emit_tr(*pend)
                                pend = (c, ib)
                    if pend is not None:
                        emit_tr(*pend)
                    # feed chunk triples {1,2,3} / {4,5,6} once all exist
                    fk = {(0, 3): (1, 2, 3), (1, 3): (4, 5, 6)}.get((wdir, k))
                    if fk == (1, 2, 3):
                        mlp_feed([(fm_hist[j], j) for j in fk], False,
                                 [(0, accr), (1, accz)])
                    elif fk is not None:
                        # last round: per-batch r adds -> sigmoid -> r*h so
                        # the d2 chains can start before the z adds finish
                        srcs = [(fm_hist[j], j) for j in fk]
                        for b in range(C):
                            mlp_feed_b(srcs, b, 0, accr)
                            r = gate_pool.tile([128, N], BF16, name="gate_t",
                                               tag="gate")
                            nc.scalar.activation(r[:], accr[:, b, :],
                                                 AF.Sigmoid,
                                                 bias=bias[:, 0:1],
                                                 scale=GATE_SCALE)
                            nc.vector.tensor_mul(fm0[:, W2P[b], :], r[:],
                                                 hb[:, b, :])
                        for b in range(C):
                            mlp_feed_b(srcs, b, 1, accz)
                    if nxt is not None:
                        cur = nxt

            xrh2 = fm0

            # ---- z and u = (1-z)*h, computed while the chains run ----
            uh = [fm_pool.tile([128, 2, N], BF16, name="fm_t", tag="fm")
                  for _ in range(2)]
            for b in range(C):
                zc = gate_pool.tile([128, N], BF16, name="gate_t", tag="gate")
                nc.scalar.activation(zc[:], accz[:, b, :], AF.Sigmoid,
                                     bias=bias[:, 3:4], scale=-GATE_SCALE)
                nc.gpsimd.tensor_mul(uh[b // 2][:, b & 1, :], zc[:],
                                     hb[:, b, :])
            for b in range(C):
                nc.scalar.activation(accz[:, b, :], accz[:, b, :], AF.Sigmoid,
                                     bias=bias[:, 1:2], scale=GATE_SCALE)

            # ---------------- diffusion 2 (n gate): Horner chains ----------------
            accn = acc_pool.tile([128, nt, 512], BF16, name="acc_t", tag="acc")

            def proj_it(ps, it, kidx, start, stop):
                """y_k projection matmuls for one node block into psum ps."""
                for b in range(C):
                    widx = 56 + (kidx * 2 + (b & 1)) * 2
                    nc.tensor.matmul(
                        ps[:, 128 * b:128 * (b + 1)],
                        xrh2[:, PB[b]:PB[b] + 2, 128 * it:128 * (it + 1)],
                        wx1[:, widx:widx + 2, :],
                        start=start, stop=stop, perf_mode=DR,
                        skip_group_check=True)

            # y0 -> accn; extra DR matmul adds the fp8 weight residual
            for it in range(nt):
                ps = psh_pool.tile([128, 512], F32, name="psh_t", tag="psh")
                for b in range(C):
                    widx = 56 + (b & 1) * 2
                    rwidx = 84 + (b & 1) * 2
                    xs = xrh2[:, PB[b]:PB[b] + 2, 128 * it:128 * (it + 1)]
                    nc.tensor.matmul(
                        ps[:, 128 * b:128 * (b + 1)], xs, wx1[:, widx:widx + 2, :],
                        start=True, stop=False, perf_mode=DR,
                        skip_group_check=True)
                    nc.tensor.matmul(
                        ps[:, 128 * b:128 * (b + 1)], xs, wx1[:, rwidx:rwidx + 2, :],
                        start=False, stop=True, perf_mode=DR,
                        skip_group_check=True)
                nc.vector.tensor_copy(accn[:, it, :], ps[:])

            def final_gate_q(q):
                """out = u + z*tanh(n) for node block q (all batches)."""
                qs = slice(512 * q, 512 * (q + 1))
                for b in range(C):
                    pt = pst_pool.tile([128, 512], BF16, name="pst_t", tag="pst")
                    for i in range(4):
                        it = 4 * q + i
                        nc.tensor.transpose(
                            pt[:, 128 * i:128 * (i + 1)],
                            accn[:, it, 128 * b:128 * (b + 1)], identb[:])
                    n_t = sg_pool.tile([128, 512], F32, name="sg_t", tag="sg")
                    nc.scalar.activation(n_t[:], pt[:], AF.Tanh,
                                         bias=bias[:, 2:3], scale=CHAIN_SCALE)
                    zn = sg_pool.tile([128, 512], F32, name="sg_t", tag="sg")
                    nc.gpsimd.tensor_mul(zn[:], n_t[:], accz[:, b, qs])
                    og = sg_pool.tile([128, 512], F32, name="og_t", tag="sg")
                    nc.gpsimd.tensor_add(og[:], zn[:], uh[b // 2][:, b & 1, qs])
                    nc.sync.dma_start(OUT[b][:, qs], og[:])

            for wdir, wt2 in ((0, wf), (1, wb)):
                # t = y_3 (projection only)
                t = xnm_pool.tile([128, nt, 512], FP8, name="tch_t", tag="xnm")
                for it in range(nt):
                    ps = psh_pool.tile([128, 512], F32, name="psh_t", tag="psh")
                    proj_it(ps, it, wdir * NHOPS + NHOPS, True, True)
                    nc.vector.tensor_scalar_mul(t[:, it, :], ps[:], CHAIN_SCALE)
                for k in range(NHOPS, 0, -1):
                    # next state = W*t (+ y_{k-1} while k>1)
                    tn = (xnm_pool.tile([128, nt, 512], FP8, name="tch_t", tag="xnm")
                          if k > 1 else None)
                    for it in range(nt):
                        ccols = slice(128 * it, 128 * (it + 1))
                        ps = psh_pool.tile([128, 512], F32, name="psh_t", tag="psh")
                        for jp in range(npair):
                            h2, j2 = divmod(2 * jp, hnt)
                            nc.tensor.matmul(
                                ps[:], wt2[h2][:, j2:j2 + 2, ccols],
                                t[:, 2 * jp:2 * jp + 2, :],
                                start=(jp == 0), stop=(jp == npair - 1 and k == 1),
                                perf_mode=DR, skip_group_check=True)
                        if k > 1:
                            proj_it(ps, it, wdir * NHOPS + k - 1, False, True)
                            if it & 1:
                                nc.scalar.activation(tn[:, it, :], ps[:],
                                                     AF.Copy, scale=CHAIN_SCALE)
                            else:
                                nc.vector.tensor_scalar_mul(
                                    tn[:, it, :], ps[:], CHAIN_SCALE)
                        else:
                            nc.vector.tensor_add(accn[:, it, :], ps[:],
                                                 accn[:, it, :])
                            if wdir == 1 and it % 4 == 3:
                                final_gate_q(it // 4)
                    if k > 1:
                        t = tn

    nc.compile()
    return nc


def _pack_gate_variants(W, scales):
    """Torch-Linear weight [128, 7*192] -> [7*2, 2, 64... ] DR variants.

    For chunk k (feature slice [k*192:(k+1)*192] = [x(64) | h(128)]) emit
    variant A (even b: planes (lo=h, hi=x-upper)) and B (odd b: planes
    (x-lower, h)), each [2, 128, 128] with rows = contraction features and
    cols = output unit. Returns [7, 2, 2, 128, 128] float32.
    """
    out = np.zeros((7, 2, 2, 128, 128), np.float32)
    for k in range(7):
        s = scales[k]
        Wx = s * W[:, k * FI:k * FI + 64].T          # [64, 128]
        Wh = s * W[:, k * FI + 64:(k + 1) * FI].T    # [128, 128]
        out[k, 0, 0] = Wh
        out[k, 0, 1, 0:64] = Wx
        out[k, 1, 0, 64:128] = Wx
        out[k, 1, 1] = Wh
    return out


_NC_CACHE = {}


def _get_nc(nt):
    if nt not in _NC_CACHE:
        _NC_CACHE[nt] = build_nc(nt)
    return _NC_CACHE[nt]


def make_in_maps(x, h_prev, W_fwd, W_bwd, Wr, br, Wz, bz, Wn, bn):
    x = np.asarray(x, np.float32)
    h_prev = np.asarray(h_prev, np.float32)
    B, N, Din = x.shape
    nt = N // 128

    def to_pmajor(a):
        # [N(j), cols] -> [128(p), nt(jt), cols] with j = jt*128 + p
        cols = a.shape[1]
        return np.ascontiguousarray(
            a.reshape(nt, 128, cols).transpose(1, 0, 2))

    WfT = to_pmajor(np.asarray(W_fwd, np.float32).T * WSCALE).astype(NP_FP8)
    WbT = to_pmajor(np.asarray(W_bwd, np.float32).T * WSCALE).astype(NP_FP8)

    d1scales = [32.0] + [4.0] * 6
    wr_v = _pack_gate_variants(np.asarray(Wr, np.float32), d1scales)
    wz_v = _pack_gate_variants(np.asarray(Wz, np.float32), d1scales)
    wn_v = _pack_gate_variants(np.asarray(Wn, np.float32), [WSCALE] * 7)
    wn8 = wn_v.astype(NP_FP8).astype(np.float32)
    dwn0 = wn_v[0] - wn8[0]                      # chunk-0 weight fp8 residual
    # wx1 [128, 56, 128]: plane ((g*7+k)*2+v)*2 + pl
    wx1 = np.concatenate([wr_v, wz_v]).reshape(14, 2, 2, 128, 128)
    wx1 = np.ascontiguousarray(
        wx1.reshape(28, 2, 128, 128).reshape(56, 128, 128)
        .transpose(1, 0, 2)).astype(NP_FP8)
    wx2 = np.concatenate(
        [wn_v.reshape(28, 128, 128), dwn0.reshape(4, 128, 128)])
    wx2 = np.ascontiguousarray(wx2.transpose(1, 0, 2)).astype(NP_FP8)
    wxb = np.stack([32.0 * np.asarray(Wr, np.float32)[:, 64:192].T,
                    32.0 * np.asarray(Wz, np.float32)[:, 64:192].T])
    wxb = np.ascontiguousarray(
        wxb.transpose(1, 0, 2)).astype(ml_dtypes.bfloat16)

    ident8 = np.eye(128, dtype=np.float32).astype(NP_FP8)
    identb = np.eye(128, dtype=np.float32).astype(ml_dtypes.bfloat16)
    biases = np.stack([np.asarray(br, np.float32),
                       np.asarray(bz, np.float32),
                       np.asarray(bn, np.float32),
                       -np.asarray(bz, np.float32)], axis=1)  # [128, 4]
    biases = np.ascontiguousarray(biases)

    ncores = B // C
    in_maps = []
    for cix in range(ncores):
        xs = x[C * cix:C * (cix + 1)]            # [C, N, 64]
        hs = h_prev[C * cix:C * (cix + 1)]       # [C, N, 128]
        # x_nm cols: [h_b0|h_b1|h_b2|h_b3|x_b0|x_b1|x_b2|x_b3]
        xnm_cols = np.concatenate(
            [hs[b] for b in range(C)] + [xs[b] for b in range(C)], axis=1)
        x_nm = to_pmajor(xnm_cols).astype(NP_FP8)
        # fm0 planes: [h0.T | x0.T;x1.T | h1.T | h2.T | x2.T;x3.T | h3.T]
        hT = [np.ascontiguousarray(hs[b].T) for b in range(C)]
        xT = [np.ascontiguousarray(xs[b].T) for b in range(C)]
        fm0 = np.stack([
            hT[0], np.concatenate([xT[0], xT[1]], axis=0), hT[1],
            hT[2], np.concatenate([xT[2], xT[3]], axis=0), hT[3],
        ], axis=1)                               # [128, 6, N]
        h_fm = np.ascontiguousarray(
            np.stack(hT, axis=1)).astype(ml_dtypes.bfloat16)  # [128, C, N]
        in_maps.append(dict(
            wfT=WfT, wbT=WbT, x_nm=x_nm,
            fm0=np.ascontiguousarray(fm0).astype(NP_FP8),
            h_fm=h_fm, wx1=wx1, wx2=wx2, wxb=wxb,
            ident=ident8, identb=identb, bias=biases))
    return in_maps, nt, ncores


def kernel(x, h_prev, W_fwd, W_bwd, Wr, br, Wz, bz, Wn, bn, _trace=False):
    in_maps, nt, ncores = make_in_maps(
        x, h_prev, W_fwd, W_bwd, Wr, br, Wz, bz, Wn, bn)
    nc = _get_nc(nt)
    res = run_bass_kernel_spmd(nc, in_maps, list(range(ncores)), trace=_trace)
    outs = [np.ascontiguousarray(res.results[c]["out_fm"].transpose(0, 2, 1))
            for c in range(ncores)]
    full = np.concatenate(outs, axis=0).astype(np.float32)
    if _trace:
        return full, res
    return full


# revision 44
# speedup vs baseline: 4.7667x; 1.0069x over previous
"""DCGRU cell on 8 Trainium2 NeuronCores.

Sharding: data-parallel over batch (B=32 -> 4 per core), adjacency + MLP
weights replicated. No collectives; host gathers per-core outputs.

Key structure (all matmuls fp8 e4m3, DoubleRow perf mode = 2 contraction
subtiles per instruction):
  - W_fwd/W_bwd cached in SBUF as WT fp8 tiles [128, 16, 2048] scaled
    x1024 (row-stochastic entries ~1e-3 are subnormal in e4m3 otherwise).
  - Diffusion 1 (r,z gates): feature-major hop outputs fm2 [128, 6, N]
    with planes [h_b0 | x_b0;x_b1 | h_b1 | h_b2 | x_b2;x_b3 | h_b3]; batch
    b's 192 features = plane pair (pb, pb+1), pb = [0,1,3,4][b]. MLP feed
    = ONE DoubleRow matmul per (gate, chunk, b, nblk) with host-packed
    weight variants (x-rows zero-padded to match the shared x-pair
    plane). Logits accumulate in SBUF bf16 at 32x scale; sigmoid applies
    scale=2^-5. Chain re-entry fm->nm via PE transposes (hops 1,2).
  - Diffusion 2 (n gate): Horner chain T = W(y1 + W(y2 + W y3)) over
    node-major state t [128, 16, 512] (cols = b*128+o). Projections
    y_k = Wn_k^T x_rh are extra matmuls accumulated into the same PSUM
    group (x_rh feature-major as stationary operand). No transposes, no
    DRAM spill. n-logits land node-major, transposed once at the end.
  - Scale chain: hop-1 stores x2^-7 (=8x true), hops 2-3 x2^-10 (=8x),
    d1 MLP weights x32 (chunk 0) / x4 (chunks 1-6); d2 weights x1024,
    chain copies 2^-10, final tanh scale 2^-10.
  - ~9 DMAs total per core (DMA dispatch, not bandwidth, dominated the
    old design).
"""

import sys
import numpy as np
import ml_dtypes

for _p in ("/opt/trn_rl_repo",):
    if _p not in sys.path:
        sys.path.insert(0, _p)

from concourse import bacc, tile, mybir  # noqa: E402
from concourse.bass_utils import run_bass_kernel_spmd  # noqa: E402

F32 = mybir.dt.float32
BF16 = mybir.dt.bfloat16
FP8 = mybir.dt.float8e4
AF = mybir.ActivationFunctionType
DR = mybir.MatmulPerfMode.DoubleRow
NP_FP8 = ml_dtypes.float8_e4m3

C = 4          # batches per core
FI = 192       # per-batch feature width (x 64 + h 128)
BF = C * FI    # 768
DH = 128
NCORES = 8
NHOPS = 3

W2P = [0, 2, 3, 5, 1, 4]   # hop psum window -> fm2 plane
PB = [0, 1, 3, 4]          # batch -> first fm2 plane of its (lo, hi) pair

WSCALE = 1024.0            # W_fwd/W_bwd host prescale
HOP1_SCALE = 2.0 ** -7     # psum -> fm2 store, hop 1 (keeps feats at 8x)
HOPK_SCALE = 2.0 ** -10    # psum -> fm2 store, hops 2+
GATE_SCALE = 2.0 ** -5     # r/z logits accumulate at 32x
CHAIN_SCALE = 2.0 ** -10   # d2 chain psum -> t store / final tanh


def build_nc(nt=16):
    """Build + compile the per-core Bass kernel. nt = node tiles (N = nt*128)."""
    N = nt * 128
    npair = nt // 2

    nc = bacc.Bacc("TRN2", target_bir_lowering=False, debug=False,
                   num_devices=NCORES)

    def din(name, shape, dt=FP8):
        return nc.dram_tensor(name, shape, dt, kind="ExternalInput").ap()

    WF = din("wfT", [128, nt, N])
    WB = din("wbT", [128, nt, N])
    XNM = din("x_nm", [128, nt, BF])
    FM0 = din("fm0", [128, 6, N])
    HB = din("h_fm", [128, C, N], BF16)
    WX1 = din("wx1", [128, 56, 128])       # ((g*7+k)*2+v)*2 -> 2 planes
    WX2 = din("wx2", [128, 32, 128])       # (k*2+v)*2 -> 2 planes; 28-31 dW0 res
    WXB = din("wxb", [128, 2, 128], BF16)  # 32*Wr0_h.T | 32*Wz0_h.T
    IDT = din("ident", [128, 128])
    IDTB = din("identb", [128, 128], BF16)
    BIAS = din("bias", [128, 4], F32)      # br | bz | bn | -bz
    OUT = nc.dram_tensor("out_fm", [C, 128, N], F32, kind="ExternalOutput").ap()

    with tile.TileContext(nc) as tc:
        with (
            tc.tile_pool(name="w", bufs=4) as w_pool,
            tc.tile_pool(name="xnm", bufs=2) as xnm_pool,
            tc.tile_pool(name="fm", bufs=3) as fm_pool,
            tc.tile_pool(name="fm0", bufs=1) as fm0_pool,
            tc.tile_pool(name="acc", bufs=2) as acc_pool,
            tc.tile_pool(name="h", bufs=1) as h_pool,
            tc.tile_pool(name="wx", bufs=1) as wx_pool,
            tc.tile_pool(name="gate", bufs=1) as gate_pool,
            tc.tile_pool(name="sg", bufs=3) as sg_pool,
            tc.tile_pool(name="const", bufs=1) as const_pool,
            tc.tile_pool(name="psh", bufs=4, space="PSUM") as psh_pool,
            tc.tile_pool(name="psm", bufs=2, space="PSUM") as psm_pool,
            tc.tile_pool(name="pst", bufs=2, space="PSUM") as pst_pool,
        ):
            # ---- one-time loads (ordered so PE can start ASAP) ----
            wxb = const_pool.tile([128, 2, 128], BF16, name="wxb_t", tag="wxb")
            nc.sync.dma_start(wxb[:], WXB[:])
            hb = h_pool.tile([128, C, N], BF16, name="h_t", tag="h")
            nc.sync.dma_start(hb[:, 0:2, :], HB[:, 0:2, :])
            nc.sync.dma_start(hb[:, 2:4, :], HB[:, 2:4, :])
            fm0 = fm0_pool.tile([128, 6, N], FP8, name="fm0_t", tag="fm0")
            nc.sync.dma_start(fm0[:], FM0[:])
            wx1 = wx_pool.tile([128, 56 + 32, 128], FP8, name="wx_t", tag="wx")
            nc.sync.dma_start(wx1[:, 0:56, :], WX1[:])
            nc.sync.dma_start(wx1[:, 56:88, :], WX2[:])
            cur = xnm_pool.tile([128, nt, BF], FP8, name="xnm_t", tag="xnm")
            nc.sync.dma_start(cur[:], XNM[:])
            hnt = nt // 2
            wf = [w_pool.tile([128, hnt, N], FP8, name="wt_t", tag="w")
                  for _ in range(2)]
            wb = [w_pool.tile([128, hnt, N], FP8, name="wt_t", tag="w")
                  for _ in range(2)]
            for hx in range(2):
                nc.sync.dma_start(wf[hx][:], WF[:, hnt * hx:hnt * (hx + 1), :])
            for hx in range(2):
                nc.sync.dma_start(wb[hx][:], WB[:, hnt * hx:hnt * (hx + 1), :])
            ident = const_pool.tile([128, 128], FP8, name="ident_t", tag="ident")
            nc.sync.dma_start(ident[:], IDT[:])
            identb = const_pool.tile([128, 128], BF16, name="identb_t", tag="identb")
            nc.sync.dma_start(identb[:], IDTB[:])
            bias = const_pool.tile([128, 4], F32, name="bias_t", tag="bias")
            nc.sync.dma_start(bias[:], BIAS[:])

            accr = acc_pool.tile([128, C, N], BF16, name="acc_t", tag="acc")
            accz = acc_pool.tile([128, C, N], BF16, name="acc_t", tag="acc")

            def hop_mm(ps, wt2, xnm2, ccols, ibs):
                """Full-contraction DR group: out[ccols-window, ibs]."""
                for jp in range(npair):
                    h2, j2 = divmod(2 * jp, hnt)
                    nc.tensor.matmul(
                        ps[:], xnm2[:, 2 * jp:2 * jp + 2, ccols],
                        wt2[h2][:, j2:j2 + 2, ibs],
                        start=(jp == 0), stop=(jp == npair - 1),
                        perf_mode=DR)

            def mlp_feed_b(srcs, b, g, acc):
                for ib in range(4):
                    nbs = slice(512 * ib, 512 * (ib + 1))
                    ps = psm_pool.tile([128, 512], F32, name="psm_t", tag="psm")
                    for s, (src, kidx) in enumerate(srcs):
                        widx = ((g * 7 + kidx) * 2 + (b & 1)) * 2
                        nc.tensor.matmul(
                            ps[:], wx1[:, widx:widx + 2, :],
                            src[:, PB[b]:PB[b] + 2, nbs],
                            start=(s == 0), stop=(s == len(srcs) - 1),
                            perf_mode=DR)
                    nc.vector.tensor_add(acc[:, b, nbs], ps[:],
                                         acc[:, b, nbs])

            def mlp_feed(srcs, init, gates, addeng=None):
                """srcs: list of (fm2, kidx) chunk pairs in one psum group.
                Accumulate bf16 logits; adds alternate DVE / Pool engines."""
                for g, acc in gates:
                    for b in range(C):
                        for ib in range(4):
                            nbs = slice(512 * ib, 512 * (ib + 1))
                            ps = psm_pool.tile([128, 512], F32, name="psm_t", tag="psm")
                            for s, (src, kidx) in enumerate(srcs):
                                widx = ((g * 7 + kidx) * 2 + (b & 1)) * 2
                                nc.tensor.matmul(
                                    ps[:], wx1[:, widx:widx + 2, :],
                                    src[:, PB[b]:PB[b] + 2, nbs],
                                    start=(s == 0), stop=(s == len(srcs) - 1),
                                    perf_mode=DR)
                            if init:
                                nc.vector.tensor_copy(acc[:, b, nbs], ps[:])
                            else:
                                nc.vector.tensor_add(acc[:, b, nbs], ps[:],
                                                     acc[:, b, nbs])

            # ---------------- diffusion 1 (r, z gates) ----------------
            # chunk-0 feed: h-part in bf16 (hb, wxb), x-part fp8 single plane
            for g, acc in ((0, accr), (1, accz)):
                for b in range(C):
                    widx = (g * 7 * 2 + (b & 1)) * 2
                    xw = widx + (0 if b & 1 else 1)
                    xpl = PB[b] if b & 1 else PB[b] + 1
                    for ib in range(4):
                        nbs = slice(512 * ib, 512 * (ib + 1))
                        ps = psm_pool.tile([128, 512], F32, name="psm_t", tag="psm")
                        nc.tensor.matmul(
                            ps[:], wxb[:, g, :], hb[:, b, nbs],
                            start=True, stop=False, skip_group_check=True)
                        nc.tensor.matmul(
                            ps[:], wx1[:, xw, :], fm0[:, xpl, nbs],
                            start=False, stop=True, skip_group_check=True)
                        nc.vector.tensor_copy(acc[:, b, nbs], ps[:])

            fm_hist = {}
            for wdir, wt2 in ((0, wf), (1, wb)):
                if wdir == 1:
                    cur = xnm_pool.tile([128, nt, BF], FP8, name="xnm_t", tag="xnm")
                    nc.sync.dma_start(cur[:], XNM[:])
                for k in range(1, NHOPS + 1):
                    cps = HOP1_SCALE if k == 1 else HOPK_SCALE
                    kidx = wdir * NHOPS + k
                    fm2 = fm_pool.tile([128, 6, N], FP8, name="fm_t", tag="fm")
                    fm_hist[kidx] = fm2
                    nxt = (xnm_pool.tile([128, nt, BF], FP8, name="xnm_t", tag="xnm")
                           if k < NHOPS else None)
                    def emit_tr(c, ib):
                        # fm -> nm re-entry (it-blocks 4*ib..4*ib+3):
                        # transpose via plain fp8 matmul against the identity
                        # (fp8 transpose mode is rejected by the compiler)
                        ccols = slice(128 * c, 128 * (c + 1))
                        pt = pst_pool.tile([128, 4, 128], F32,
                                           name="pst_t", tag="pst")
                        for i in range(4):
                            it = 4 * ib + i
                            nc.tensor.matmul(
                                pt[:, i, :],
                                fm2[:, W2P[c], 128 * it:128 * (it + 1)],
                                ident[:], start=True, stop=True)
                        nc.scalar.activation(
                            nxt[:, 4 * ib:4 * ib + 4, ccols], pt[:], AF.Copy)

                    pend = None
                    for c in range(6):
                        ccols = slice(128 * c, 128 * (c + 1))
                        for ib in range(4):
                            ibs = slice(512 * ib, 512 * (ib + 1))
                            ps = psh_pool.tile([128, 512], F32, name="psh_t", tag="psh")
                            hop_mm(ps, wt2, cur, ccols, ibs)
                            last = (wdir, k) == (1, NHOPS)
                            eng = (nc.vector if last or (c * 4 + ib) & 1
                                   else nc.scalar)
                            if eng is nc.vector:
                                nc.vector.tensor_scalar_mul(
                                    fm2[:, W2P[c], ibs], ps[:], cps)
                            else:
                                nc.scalar.activation(
                                    fm2[:, W2P[c], ibs], ps[:], AF.Copy,
                                    scale=cps)
                            if nxt is not None:
                                if pend is not None:
                                    emit_tr(*pend)
                                pend = (c, ib)
                    if pend is not None:
                        emit_tr(*pend)
                    # feed chunk triples {1,2,3} / {4,5,6} once all exist
                    fk = {(0, 3): (1, 2, 3), (1, 3): (4, 5, 6)}.get((wdir, k))
                    if fk == (1, 2, 3):
                        mlp_feed([(fm_hist[j], j) for j in fk], False,
                                 [(0, accr), (1, accz)])
                    elif fk is not None:
                        # last round: per-batch r adds -> sigmoid -> r*h so
                        # the d2 chains can start before the z adds finish
                        srcs = [(fm_hist[j], j) for j in fk]
                        for b in range(C):
                            mlp_feed_b(srcs, b, 0, accr)
                            r = gate_pool.tile([128, N], BF16, name="gate_t",
                                               tag="gate")
                            nc.scalar.activation(r[:], accr[:, b, :],
                                                 AF.Sigmoid,
                                                 bias=bias[:, 0:1],
                                                 scale=GATE_SCALE)
                            nc.vector.tensor_mul(fm0[:, W2P[b], :], r[:],
                                                 hb[:, b, :])
                        for b in range(C):
                            mlp_feed_b(srcs, b, 1, accz)
                    if nxt is not None:
                        cur = nxt

            xrh2 = fm0

            # ---- z and u = (1-z)*h, computed while the chains run ----
            uh = [fm_pool.tile([128, 2, N], BF16, name="fm_t", tag="fm")
                  for _ in range(2)]
            for b in range(C):
                zc = gate_pool.tile([128, N], BF16, name="gate_t", tag="gate")
                nc.scalar.activation(zc[:], accz[:, b, :], AF.Sigmoid,
                                     bias=bias[:, 3:4], scale=-GATE_SCALE)
                nc.gpsimd.tensor_mul(uh[b // 2][:, b & 1, :], zc[:],
                                     hb[:, b, :])
            for b in range(C):
                nc.scalar.activation(accz[:, b, :], accz[:, b, :], AF.Sigmoid,
                                     bias=bias[:, 1:2], scale=GATE_SCALE)

            # ---------------- diffusion 2 (n gate): Horner chains ----------------
            # n-logit accumulator in FM orientation [o, b, n]: the k=1 chain
            # step emits FM directly (t as stationary operand), so no final
            # transposes are needed
            accn = acc_pool.tile([128, C, N], BF16, name="acc_t", tag="acc")

            def proj_it(ps, it, kidx, start, stop):
                """y_k projection matmuls for one node block into psum ps."""
                for b in range(C):
                    widx = 56 + (kidx * 2 + (b & 1)) * 2
                    nc.tensor.matmul(
                        ps[:, 128 * b:128 * (b + 1)],
                        xrh2[:, PB[b]:PB[b] + 2, 128 * it:128 * (it + 1)],
                        wx1[:, widx:widx + 2, :],
                        start=start, stop=stop, perf_mode=DR,
                        skip_group_check=True)

            # y0 -> accn (FM); extra DR matmul adds the fp8 weight residual
            for b in range(C):
                widx = 56 + (b & 1) * 2
                rwidx = 84 + (b & 1) * 2
                for ib in range(4):
                    nbs = slice(512 * ib, 512 * (ib + 1))
                    ps = psh_pool.tile([128, 512], F32, name="psh_t", tag="psh")
                    xs = xrh2[:, PB[b]:PB[b] + 2, nbs]
                    nc.tensor.matmul(ps[:], wx1[:, widx:widx + 2, :], xs,
                                     start=True, stop=False, perf_mode=DR,
                                     skip_group_check=True)
                    nc.tensor.matmul(ps[:], wx1[:, rwidx:rwidx + 2, :], xs,
                                     start=False, stop=True, perf_mode=DR,
                                     skip_group_check=True)
                    nc.vector.tensor_copy(accn[:, b, nbs], ps[:])

            def final_gate_bi(b, nbs):
                """out = u + z*tanh(n) for one (batch, node block)."""
                n_t = sg_pool.tile([128, 512], F32, name="sg_t", tag="sg")
                nc.scalar.activation(n_t[:], accn[:, b, nbs], AF.Tanh,
                                     bias=bias[:, 2:3], scale=CHAIN_SCALE)
                zn = sg_pool.tile([128, 512], F32, name="sg_t", tag="sg")
                nc.gpsimd.tensor_mul(zn[:], n_t[:], accz[:, b, nbs])
                og = sg_pool.tile([128, 512], F32, name="og_t", tag="sg")
                nc.gpsimd.tensor_add(og[:], zn[:], uh[b // 2][:, b & 1, nbs])
                nc.sync.dma_start(OUT[b][:, nbs], og[:])

            for wdir, wt2 in ((0, wf), (1, wb)):
                # t = y_3 (projection only)
                t = xnm_pool.tile([128, nt, 512], FP8, name="tch_t", tag="xnm")
                for it in range(nt):
                    ps = psh_pool.tile([128, 512], F32, name="psh_t", tag="psh")
                    proj_it(ps, it, wdir * NHOPS + NHOPS, True, True)
                    nc.vector.tensor_scalar_mul(t[:, it, :], ps[:], CHAIN_SCALE)
                for k in range(NHOPS, 1, -1):
                    # next state = W*t + y_{k-1}
                    tn = xnm_pool.tile([128, nt, 512], FP8, name="tch_t", tag="xnm")
                    for it in range(nt):
                        ccols = slice(128 * it, 128 * (it + 1))
                        ps = psh_pool.tile([128, 512], F32, name="psh_t", tag="psh")
                        for jp in range(npair):
                            h2, j2 = divmod(2 * jp, hnt)
                            nc.tensor.matmul(
                                ps[:], wt2[h2][:, j2:j2 + 2, ccols],
                                t[:, 2 * jp:2 * jp + 2, :],
                                start=(jp == 0), stop=False,
                                perf_mode=DR, skip_group_check=True)
                        proj_it(ps, it, wdir * NHOPS + k - 1, False, True)
                        if it & 1:
                            nc.scalar.activation(tn[:, it, :], ps[:],
                                                 AF.Copy, scale=CHAIN_SCALE)
                        else:
                            nc.vector.tensor_scalar_mul(
                                tn[:, it, :], ps[:], CHAIN_SCALE)
                    t = tn
                # last step emits T in FM orientation (t1 as stationary)
                for b in range(C):
                    bcols = slice(128 * b, 128 * (b + 1))
                    for ib in range(4):
                        nbs = slice(512 * ib, 512 * (ib + 1))
                        ps = psh_pool.tile([128, 512], F32, name="psh_t", tag="psh")
                        hop_mm(ps, wt2, t, bcols, nbs)
                        nc.vector.tensor_add(accn[:, b, nbs], ps[:],
                                             accn[:, b, nbs])
                        if wdir == 1:
                            final_gate_bi(b, nbs)

    nc.compile()
    return nc


def _pack_gate_variants(W, scales):
    """Torch-Linear weight [128, 7*192] -> [7*2, 2, 64... ] DR variants.

    For chunk k (feature slice [k*192:(k+1)*192] = [x(64) | h(128)]) emit
    variant A (even b: planes (lo=h, hi=x-upper)) and B (odd b: planes
    (x-lower, h)), each [2, 128, 128] with rows = contraction features and
    cols = output unit. Returns [7, 2, 2, 128, 128] float32.
    """
    out = np.zeros((7, 2, 2, 128, 128), np.float32)
    for k in range(7):
        s = scales[k]
        Wx = s * W[:, k * FI:k * FI + 64].T          # [64, 128]
        Wh = s * W[:, k * FI + 64:(k + 1) * FI].T    # [128, 128]
        out[k, 0, 0] = Wh
        out[k, 0, 1, 0:64] = Wx
        out[k, 1, 0, 64:128] = Wx
        out[k, 1, 1] = Wh
    return out


_NC_CACHE = {}


def _get_nc(nt):
    if nt not in _NC_CACHE:
        _NC_CACHE[nt] = build_nc(nt)
    return _NC_CACHE[nt]


def make_in_maps(x, h_prev, W_fwd, W_bwd, Wr, br, Wz, bz, Wn, bn):
    x = np.asarray(x, np.float32)
    h_prev = np.asarray(h_prev, np.float32)
    B, N, Din = x.shape
    nt = N // 128

    def to_pmajor(a):
        # [N(j), cols] -> [128(p), nt(jt), cols] with j = jt*128 + p
        cols = a.shape[1]
        return np.ascontiguousarray(
            a.reshape(nt, 128, cols).transpose(1, 0, 2))

    WfT = to_pmajor(np.asarray(W_fwd, np.float32).T * WSCALE).astype(NP_FP8)
    WbT = to_pmajor(np.asarray(W_bwd, np.float32).T * WSCALE).astype(NP_FP8)

    d1scales = [32.0] + [4.0] * 6
    wr_v = _pack_gate_variants(np.asarray(Wr, np.float32), d1scales)
    wz_v = _pack_gate_variants(np.asarray(Wz, np.float32), d1scales)
    wn_v = _pack_gate_variants(np.asarray(Wn, np.float32), [WSCALE] * 7)
    wn8 = wn_v.astype(NP_FP8).astype(np.float32)
    dwn0 = wn_v[0] - wn8[0]                      # chunk-0 weight fp8 residual
    # wx1 [128, 56, 128]: plane ((g*7+k)*2+v)*2 + pl
    wx1 = np.concatenate([wr_v, wz_v]).reshape(14, 2, 2, 128, 128)
    wx1 = np.ascontiguousarray(
        wx1.reshape(28, 2, 128, 128).reshape(56, 128, 128)
        .transpose(1, 0, 2)).astype(NP_FP8)
    wx2 = np.concatenate(
        [wn_v.reshape(28, 128, 128), dwn0.reshape(4, 128, 128)])
    wx2 = np.ascontiguousarray(wx2.transpose(1, 0, 2)).astype(NP_FP8)
    wxb = np.stack([32.0 * np.asarray(Wr, np.float32)[:, 64:192].T,
                    32.0 * np.asarray(Wz, np.float32)[:, 64:192].T])
    wxb = np.ascontiguousarray(
        wxb.transpose(1, 0, 2)).astype(ml_dtypes.bfloat16)

    ident8 = np.eye(128, dtype=np.float32).astype(NP_FP8)
    identb = np.eye(128, dtype=np.float32).astype(ml_dtypes.bfloat16)
    biases = np.stack([np.asarray(br, np.float32),
                       np.asarray(bz, np.float32),
                       np.asarray(bn, np.float32),
                       -np.asarray(bz, np.float32)], axis=1)  # [128, 4]
    biases = np.ascontiguousarray(biases)

    ncores = B // C
    in_maps = []
    for cix in range(ncores):
        xs = x[C * cix:C * (cix + 1)]            # [C, N, 64]
        hs = h_prev[C * cix:C * (cix + 1)]       # [C, N, 128]
        # x_nm cols: [h_b0|h_b1|h_b2|h_b3|x_b0|x_b1|x_b2|x_b3]
        xnm_cols = np.concatenate(
            [hs[b] for b in range(C)] + [xs[b] for b in range(C)], axis=1)
        x_nm = to_pmajor(xnm_cols).astype(NP_FP8)
        # fm0 planes: [h0.T | x0.T;x1.T | h1.T | h2.T | x2.T;x3.T | h3.T]
        hT = [np.ascontiguousarray(hs[b].T) for b in range(C)]
        xT = [np.ascontiguousarray(xs[b].T) for b in range(C)]
        fm0 = np.stack([
            hT[0], np.concatenate([xT[0], xT[1]], axis=0), hT[1],
            hT[2], np.concatenate([xT[2], xT[3]], axis=0), hT[3],
        ], axis=1)                               # [128, 6, N]
        h_fm = np.ascontiguousarray(
            np.stack(hT, axis=1)).astype(ml_dtypes.bfloat16)  # [128, C, N]
        in_maps.append(dict(
            wfT=WfT, wbT=WbT, x_nm=x_nm,
            fm0=np.ascontiguousarray(fm0).astype(NP_FP8),
            h_fm=h_fm, wx1=wx1, wx2=wx2, wxb=wxb,
            ident=ident8, identb=identb, bias=biases))
    return in_maps, nt, ncores


def kernel(x, h_prev, W_fwd, W_bwd, Wr, br, Wz, bz, Wn, bn, _trace=False):
    in_maps, nt, ncores = make_in_maps(
        x, h_prev, W_fwd, W_bwd, Wr, br, Wz, bz, Wn, bn)
    nc = _get_nc(nt)
    res = run_bass_kernel_spmd(nc, in_maps, list(range(ncores)), trace=_trace)
    outs = [np.ascontiguousarray(res.results[c]["out_fm"].transpose(0, 2, 1))
            for c in range(ncores)]
    full = np.concatenate(outs, axis=0).astype(np.float32)
    if _trace:
        return full, res
    return full
